# revision 33
# baseline (speedup 1.0000x reference)
"""AttentionBlock Trainium2 kernel (v2).

Data-parallel: one batch element per NeuronCore (8 cores, no collectives).

Per core, with xr = x[b] viewed as [C, S] (C=512 channels, S=1024 tokens):
    QT = wq^T @ xr + bq   -> [D, S]  (d on partitions; head h = rows 64h..64h+63)
    KT = wk^T @ xr + bk   -> [D, S]
    V  = xr^T @ wv + bv   -> [S, D]  (tokens on partitions)
    per head h: ET[j, i] = KT_h^T . QT_h            (keys j on psum partitions)
                E = exp(ET / sqrt(C))               (no max-subtract; |args| < ~3)
                O'T[d, i] = sum_j V[j, d] E[j, i];  Z[i] = sum_j E[j, i]
                OT[d, i] = O'T[d, i] / Z[i]
    y = wo^T @ OT + bo + xr   -> [C, S]

All matmul operands are fp16 (cast host-side; DMA moves 2B/elem). PSUM stays
fp32. Heads run in pairs (2t, 2t+1): the pair's K rows live in SBUF partitions
0..63 / 64..127, so the two K=64 energy matmuls run concurrently in disjoint
PE row groups.

V' stationary layout per (jc, h): 128 columns = [ones(64) | V(64)]. The AV
matmul therefore lands Z = sum_j E[j, i] REPLICATED on psum partitions 0..63
(and O' on 64..127) for every head, so the softmax normalization is just
  rb[0:64, h] = 1/ps_av[0:64, h]   (DVE reciprocal, 64 lanes)
  OT[64h..]   = ps_av[64:128, h] * rb[0:64, h]
with no cross-partition broadcast (the old gpsimd partition_broadcast is gone).

exp is split between ScalarE (exact, table-based) and the Vector engine
(Schraudolph bit-trick: exp(s*x) ~= bitcast_f16(u16(x*A + B)), one full-rate
tensor_scalar op; |rel err| < ~2%, which drowns in the softmax average).
ATTN_DVE_EXP selects how many of the 8 key-chunk units per (t, i) go to DVE.

Pipelining: energy pair -> exp -> (AV deferred one unit so the in-order PE
stream never waits on this unit's exp). PSUM (8 banks): energy units
double-buffered (4), one AV accumulator (2), QK/vproj/warm on a separate aux
buffer (2) so the hoisted next-pair QK matmuls never displace the energy
rotation. The previous half's norm is flushed to the DVE queue at each half
start (required: with a single AV buffer, a DVE-exp op queued ahead of the
norm would deadlock the PE FIFO). QK for head-pair t+1 is emitted mid-way
through t's stream to fill PE slack; the final projection's cc=3 accumulator
(the one gated on the last norm) is emitted after 18 cover matmuls.
"""

import math
import os

import numpy as np

B = 8
C = 512
S = 1024  # 32*32 tokens
NH = 8
HD = 64
P = 128
CC = C // P  # 4 contraction chunks of 128
NI = 2  # S split into 2 chunks of 512 for matmul free dim
SC = S // P  # 8 key chunks of 128

DVE_EXP = int(os.environ.get("ATTN_DVE_EXP", "1"))  # of 8 units per (t, i)
DIV_NORM = os.environ.get("ATTN_DIV_NORM", "0") == "1"
SCHRAUD_C = float(os.environ.get("ATTN_SCHRAUD_C", "44.75"))
N_WARM = int(os.environ.get("ATTN_WARM", "28"))

# Which key-chunk units of each (t, i) run exp on DVE. Positioned mid-half,
# right after the previous half's norm chain (2.7us) drains from the DVE
# FIFO — an exp-DVE op queued behind other DVE work stalls the 2-deep
# energy-tile rotation and starves ScalarE.
_DVE_UNITS = {0: (), 1: (4,), 2: (4, 5), 3: (3, 4, 5), 4: (3, 4, 5, 6)}.get(
    DVE_EXP, tuple(range(DVE_EXP))
)


def _emit(nc, tc, mybir, aps):
    import contextlib

    F32 = mybir.dt.float32
    F16 = mybir.dt.float16
    U16 = mybir.dt.uint16
    MULT = mybir.AluOpType.mult
    ADD = mybir.AluOpType.add
    DIV = mybir.AluOpType.divide
    EXP = mybir.ActivationFunctionType.Exp
    softmax_scale = 1.0 / math.sqrt(C)
    ONE_F16 = 0x3C00

    # Schraudolph constants for f16 bit-pattern exp of (softmax_scale * x):
    # u16(x * scale * 1024/ln2 + (15*1024 - c))
    SCH_A = softmax_scale * 1024.0 / math.log(2.0)
    SCH_B = 15.0 * 1024.0 - SCHRAUD_C

    xb, wq, bq, wk, bk, wv, bv, wo, bo, y = (
        aps[k] for k in ("xb", "wq", "bq", "wk", "bk", "wv", "bv", "wo", "bo", "y")
    )
    ident, bo16 = aps["ident"], aps["bo16"]
    xb_r = xb.rearrange("(cc p) s -> p cc s", p=P)
    y_r = y.rearrange("(cc p) s -> p cc s", p=P)
    wq_r = wq.rearrange("(cc p) d -> p cc d", p=P)
    wk_r = wk.rearrange("(cc p) d -> p cc d", p=P)
    wv_r = wv.rearrange("(cc p) d -> p cc d", p=P)
    wo_r = wo.rearrange("(dc p) c -> p dc c", p=P)
    bq_r = bq.rearrange("(dc p) -> p dc", p=P)
    bk_r = bk.rearrange("(dc p) -> p dc", p=P)
    bo_r = bo.rearrange("(cc p) -> p cc", p=P)

    with contextlib.ExitStack() as ctx:
        singles = ctx.enter_context(tc.tile_pool(name="singles", bufs=1))
        qkpool = ctx.enter_context(tc.tile_pool(name="qk", bufs=2))
        etpool = ctx.enter_context(tc.tile_pool(name="et", bufs=8))
        rbpool = ctx.enter_context(tc.tile_pool(name="rb", bufs=4))
        tmppool = ctx.enter_context(tc.tile_pool(name="tmp", bufs=4))
        # PSUM bank budget (8 banks): energy 2 tiles x 2 banks, AV 1 x 2,
        # QK/vproj/warm (aux) 1 x 2. Keeping aux OUT of the energy pool means
        # the hoisted QK matmuls never displace the energy-tile rotation.
        pse = ctx.enter_context(tc.tile_pool(name="pse", bufs=2, space="PSUM"))
        psav = ctx.enter_context(tc.tile_pool(name="psav", bufs=1, space="PSUM"))
        psaux = ctx.enter_context(tc.tile_pool(name="psaux", bufs=1, space="PSUM"))

        # ---- input DMAs, spread across queues, first-needed first ----
        # sync: xb query-half 0; vector: xb half 1 (DVE idle at start)
        # scalar: bq/bk + wq/wk (t=0 columns first); gpsimd: bv/wv + bo/wo
        xb_sb = singles.tile([P, CC, S], F16)
        bq_sb = singles.tile([P, CC], F32)
        bk_sb = singles.tile([P, CC], F32)
        bo_sb = singles.tile([P, CC], F32)
        bv_sb = singles.tile([1, C], F16)
        wq_sb = singles.tile([P, CC, C], F16)
        wk_sb = singles.tile([P, CC, C], F16)
        wv_sb = singles.tile([P, CC, C], F16)
        wo_sb = singles.tile([P, CC, C], F16)

        # xb per-cc chunks (2KB contiguous rows) in the order the t=0 QK
        # accumulation consumes them; wq/wk per-cc interleaved likewise.
        for cc in range(CC):
            nc.sync.dma_start(out=xb_sb[:, cc], in_=xb_r[:, cc])
        nc.scalar.dma_start(out=wq_sb[:, 0], in_=wq_r[:, 0])
        nc.scalar.dma_start(out=wk_sb[:, 0], in_=wk_r[:, 0])
        nc.scalar.dma_start(out=bq_sb, in_=bq_r)
        nc.scalar.dma_start(out=bk_sb, in_=bk_r)
        for cc in range(1, CC):
            nc.scalar.dma_start(out=wq_sb[:, cc], in_=wq_r[:, cc])
            nc.scalar.dma_start(out=wk_sb[:, cc], in_=wk_r[:, cc])
        nc.gpsimd.dma_start(out=bv_sb, in_=bv[None, :])
        nc.gpsimd.dma_start(out=wv_sb, in_=wv_r)
        nc.gpsimd.dma_start(out=bo_sb, in_=bo_r)
        nc.gpsimd.dma_start(out=wo_sb, in_=wo_r)

        ident_sb = singles.tile([P, P], F16)
        nc.gpsimd.dma_start(out=ident_sb, in_=ident)
        bo16_sb = singles.tile([1, C], F16)
        nc.gpsimd.dma_start(out=bo16_sb, in_=bo16[None, :])
        ones_row = singles.tile([1, P], F16)
        nc.vector.memset(ones_row.bitcast(U16), ONE_F16)
        ones512 = singles.tile([1, 512], F16)
        nc.vector.memset(ones512.bitcast(U16), ONE_F16)

        # V' layout: 128 columns per (jc, h): [ones(64) | V(64)]. The AV matmul
        # (M=128) replicates Z = sum_j E[j, i] on psum rows 0..63 and puts O'
        # on rows 64..127 for every head. The ones halves are memset per key
        # chunk on GpSimd (idle engine; keeps the DVE queue clear for the t=0
        # bias adds); the V copies overwrite cols 64..127.
        Vp = singles.tile([P, SC, NH, P], F16)
        for sc in range(SC):
            nc.gpsimd.memset(Vp[:, sc, :, 0:64].bitcast(U16), ONE_F16)

        # PE warm-up on zeros while input DMAs land (HAM clock-gate at 8/8
        # before real matmuls start)
        warm = singles.tile([P, 512], F16)
        nc.vector.memset(warm.bitcast(U16), 0)
        ps_w = psaux.tile([P, 2, 512], F32, tag="aux")
        for _ in range(N_WARM):
            nc.tensor.matmul(ps_w[:, 0], warm[:, 0:128], warm)

        OTs = [singles.tile([P, S], F16, tag=f"ot{t}", name=f"ot{t}") for t in range(CC)]

        def emit_v_projection_chunk(sc):
            # V[s, d] = xr^T @ wv + bv for one token chunk
            ps_v = psaux.tile([P, 2, 512], F32, tag="aux")
            for cc in range(CC):
                nc.tensor.matmul(
                    ps_v[:, 0],
                    xb_sb[:, cc, sc * P : (sc + 1) * P],
                    wv_sb[:, cc],
                    start=(cc == 0),
                    stop=False,
                )
            nc.tensor.matmul(ps_v[:, 0], ones_row, bv_sb, start=False, stop=True)
            psv_r = ps_v[:, 0].rearrange("p (h d) -> p h d", h=NH)
            nc.vector.tensor_copy(out=Vp[:, sc, :, 64:128], in_=psv_r)

        pending_norm = [None]
        pending_av = []

        def flush_av(depth=0):
            while len(pending_av) > depth:
                pending_av.pop(0)()

        def flush_norm():
            if pending_norm[0] is not None:
                pending_norm[0]()
                pending_norm[0] = None

        qk_tiles = {}

        def emit_qk(t):
            # QT/KT for heads (2t, 2t+1)
            qt = qkpool.tile([P, S], F16, tag="qt", name=f"qt{t}")
            kt = qkpool.tile([P, S], F16, tag="kt", name=f"kt{t}")
            qk_tiles[t] = (qt, kt)
            for i in range(NI):
                sl = slice(i * 512, (i + 1) * 512)
                ps_p = psaux.tile([P, 2, 512], F32, tag="aux")
                for cc in range(CC):
                    xsl = xb_sb[:, cc, sl]
                    nc.tensor.matmul(
                        ps_p[:, 0],
                        wq_sb[:, cc, t * P : (t + 1) * P],
                        xsl,
                        start=(cc == 0),
                        stop=(cc == CC - 1),
                    )
                    nc.tensor.matmul(
                        ps_p[:, 1],
                        wk_sb[:, cc, t * P : (t + 1) * P],
                        xsl,
                        start=(cc == 0),
                        stop=(cc == CC - 1),
                    )
                nc.vector.tensor_scalar_add(qt[:, sl], ps_p[:, 0], bq_sb[:, t : t + 1])
                nc.vector.tensor_scalar_add(kt[:, sl], ps_p[:, 1], bk_sb[:, t : t + 1])

        # ---- per head-pair t ----
        # QK for t+1 is emitted mid-way through t's first query-half, where it
        # fills the PE slack of the ScalarE-bound exp stream (instead of
        # stalling ACT for ~4.7us at every head-pair seam).
        emit_qk(0)
        for t in range(CC):
            qt, kt = qk_tiles.pop(t)
            # energy -> exp -> AV, pipelined per (query-half i, key-chunk jc).
            h0, h1 = 2 * t, 2 * t + 1
            for i in range(NI):
                sl = slice(i * 512, (i + 1) * 512)
                # The previous half's norm must be emitted to the DVE queue
                # before this half's DVE-exp units: with a single AV buffer,
                # an exp-DVE op queued ahead of the norm would deadlock the
                # PE FIFO (av waits buffer <- norm waits exp <- energy behind
                # the stalled av).
                flush_norm()
                ps_av = psav.tile([P, 2, 512], F32, tag="av")  # h0, h1
                for jc in range(SC):
                    if t == 0 and i == 0:
                        emit_v_projection_chunk(jc)
                    hoist = (1, 6) if t == 0 else (0, 6)
                    if (i, jc) == hoist and t < CC - 1:
                        emit_qk(t + 1)
                    k0 = kt[0:64, jc * P : (jc + 1) * P]
                    k1 = kt[64:128, jc * P : (jc + 1) * P]
                    first, last = jc == 0, jc == SC - 1
                    ps_e = pse.tile([P, 2, 512], F32, tag="e")  # head-major
                    nc.tensor.matmul(ps_e[:, 0], k0, qt[0:64, sl])
                    nc.tensor.matmul(ps_e[:, 1], k1, qt[64:128, sl])
                    et = etpool.tile([P, 2, 512], F16, tag="et")
                    if jc in _DVE_UNITS:
                        # Schraudolph f16 bit-pattern exp on DVE
                        nc.vector.tensor_scalar(
                            et.bitcast(U16), ps_e, SCH_A, SCH_B, MULT, ADD
                        )
                    else:
                        nc.scalar.activation(
                            out=et, in_=ps_e, func=EXP, scale=softmax_scale
                        )
                    # AV emitted one unit late so the in-order PE stream never
                    # waits on this unit's exp
                    flush_av(depth=1)

                    def av(ps_av=ps_av, jc=jc, et=et, h0=h0, h1=h1,
                           first=first, last=last):
                        nc.tensor.matmul(
                            ps_av[:, 0], Vp[:, jc, h0], et[:, 0],
                            start=first, stop=last,
                        )
                        nc.tensor.matmul(
                            ps_av[:, 1], Vp[:, jc, h1], et[:, 1],
                            start=first, stop=last,
                        )

                    pending_av.append(av)

                flush_av()

                def norm(t=t, sl=sl, ps_av=ps_av):
                    # Z replicated on psum rows 0..63 (ones half of V'); O' on
                    # rows 64..127. Reciprocal + multiply read partition-
                    # aligned rows; builtin DVE ops may cross partition bases.
                    if DIV_NORM:
                        nc.vector.tensor_tensor(
                            OTs[t][0:64, sl], ps_av[64:128, 0], ps_av[0:64, 0], DIV
                        )
                        nc.vector.tensor_tensor(
                            OTs[t][64:128, sl], ps_av[64:128, 1], ps_av[0:64, 1], DIV
                        )
                    else:
                        rb = rbpool.tile([64, 2, 512], F32, tag="rb")
                        nc.vector.reciprocal_approx_fast(out=rb, in_=ps_av[0:64])
                        nc.vector.tensor_tensor(
                            OTs[t][0:64, sl], ps_av[64:128, 0], rb[:, 0], MULT
                        )
                        nc.vector.tensor_tensor(
                            OTs[t][64:128, sl], ps_av[64:128, 1], rb[:, 1], MULT
                        )

                pending_norm[0] = norm

        # ---- final projection + bias + residual ----
        # All four cc accumulators live at once (two from each drained pool),
        # so the 32 matmuls run back-to-back; dc=3 waits only on the deferred
        # t=3 normalization, which overlaps dc=0..2.
        # cc=0..2 accumulators have no norm dependency (their psum buffers are
        # freed by exp / the early norm); cc=3's buffer waits on the deferred
        # t=3 i=1 norm, so its matmuls are emitted after 18 cover matmuls.
        flush_norm()
        ps_fs = [
            pse.tile([P, 2, 512], F32, tag="e", name="psf0"),
            pse.tile([P, 2, 512], F32, tag="e", name="psf1"),
            psaux.tile([P, 2, 512], F32, tag="aux", name="psf2"),
            psav.tile([P, 2, 512], F32, tag="av", name="psf3"),
        ]

        def fmm(dc, cc, start, stop):
            wo_sl = wo_sb[:, dc, cc * P : (cc + 1) * P]
            for i in range(NI):
                sl = slice(i * 512, (i + 1) * 512)
                nc.tensor.matmul(
                    ps_fs[cc][:, i], wo_sl, OTs[dc][:, sl], start=start, stop=stop,
                )

        # For cc in ACT_CCS the residual x and bias bo are folded into the
        # psum via identity / ones-row matmuls, so the fp32->fp16 conversion
        # is a plain Copy that runs on the (idle by now) ScalarE, in parallel
        # with the other chunks' scalar_tensor_tensor on the Vector engine.
        ACT_CCS = (1, 3)
        COPY = mybir.ActivationFunctionType.Copy
        for dc in range(CC - 1):
            for cc in range(CC - 1):
                fmm(dc, cc, dc == 0, False)
        for dc in range(CC - 1):
            fmm(dc, 3, dc == 0, False)
        for cc in ACT_CCS:
            for i in range(NI):
                sl = slice(i * 512, (i + 1) * 512)
                nc.tensor.matmul(
                    ps_fs[cc][:, i], ident_sb, xb_sb[:, cc, sl],
                    start=False, stop=False,
                )
                nc.tensor.matmul(
                    ps_fs[cc][:, i],
                    bo16_sb[0:1, cc * P : (cc + 1) * P],
                    ones512,
                    start=False, stop=False,
                )
        out_q = [nc.sync, nc.scalar, nc.gpsimd]
        for cc in range(CC):
            fmm(CC - 1, cc, False, True)
            for i in range(NI):
                sl = slice(i * 512, (i + 1) * 512)
                tmp = tmppool.tile([P, 512], F16, tag="tmp")
                if cc in ACT_CCS:
                    nc.scalar.activation(out=tmp, in_=ps_fs[cc][:, i], func=COPY)
                else:
                    nc.vector.scalar_tensor_tensor(
                        out=tmp,
                        in0=ps_fs[cc][:, i],
                        scalar=bo_sb[:, cc : cc + 1],
                        in1=xb_sb[:, cc, sl],
                        op0=ADD,
                        op1=ADD,
                    )
                out_q[(2 * cc + i) % 3].dma_start(out=y_r[:, cc, sl], in_=tmp)


_NC_CACHE = {}


def _build():
    key = (DVE_EXP, DIV_NORM, SCHRAUD_C, N_WARM)
    if key in _NC_CACHE:
        return _NC_CACHE[key]
    import concourse.bacc as bacc
    import concourse.mybir as mybir
    import concourse.tile as tile

    F32 = mybir.dt.float32
    F16 = mybir.dt.float16
    nc = bacc.Bacc("TRN2", target_bir_lowering=False, debug=False)
    aps = {}
    aps["xb"] = nc.dram_tensor("xb", (C, S), F16, kind="ExternalInput").ap()
    for name in ("wq", "wk", "wv", "wo"):
        aps[name] = nc.dram_tensor(name, (C, C), F16, kind="ExternalInput").ap()
    for name in ("bq", "bk", "bo"):
        aps[name] = nc.dram_tensor(name, (C,), F32, kind="ExternalInput").ap()
    aps["bv"] = nc.dram_tensor("bv", (C,), F16, kind="ExternalInput").ap()
    aps["ident"] = nc.dram_tensor("ident", (P, P), F16, kind="ExternalInput").ap()
    aps["bo16"] = nc.dram_tensor("bo16", (C,), F16, kind="ExternalInput").ap()
    # y in f16: halves the output DMA tail; the extra ~5e-4 rounding is far
    # inside the tolerance. The host upcasts back to f32.
    aps["y"] = nc.dram_tensor("y", (C, S), F16, kind="ExternalOutput").ap()
    with tile.TileContext(nc) as tc:
        _emit(nc, tc, mybir, aps)
    nc.compile()
    _NC_CACHE[key] = nc
    return nc


def prepare_in_maps(x, wq, bq, wk, bk, wv, bv, wo, bo):
    """Host-side prep: cast activations/weights to f16, shard x per core."""
    x = np.asarray(x, dtype=np.float32).reshape(B, C, S)
    weights = {
        "wq": np.ascontiguousarray(np.asarray(wq, dtype=np.float16)),
        "bq": np.ascontiguousarray(np.asarray(bq, dtype=np.float32)),
        "wk": np.ascontiguousarray(np.asarray(wk, dtype=np.float16)),
        "bk": np.ascontiguousarray(np.asarray(bk, dtype=np.float32)),
        "wv": np.ascontiguousarray(np.asarray(wv, dtype=np.float16)),
        "bv": np.ascontiguousarray(np.asarray(bv, dtype=np.float16)),
        "wo": np.ascontiguousarray(np.asarray(wo, dtype=np.float16)),
        "bo": np.ascontiguousarray(np.asarray(bo, dtype=np.float32)),
        "bo16": np.ascontiguousarray(np.asarray(bo, dtype=np.float16)),
        "ident": np.eye(P, dtype=np.float16),
    }
    return [
        {"xb": np.ascontiguousarray(x[b].astype(np.float16)), **weights}
        for b in range(B)
    ]


def kernel(x, wq, bq, wk, bk, wv, bv, wo, bo):
    from concourse import bass_utils

    nc = _build()
    in_maps = prepare_in_maps(x, wq, bq, wk, bk, wv, bv, wo, bo)
    res = bass_utils.run_bass_kernel_spmd(nc, in_maps, core_ids=list(range(B)))
    out = np.stack([r["y"].astype(np.float32) for r in res.results])
    return out.reshape(B, C, 32, 32)


# revision 34
# speedup vs baseline: 1.0257x; 1.0257x over previous
"""AttentionBlock Trainium2 kernel (v2).

Data-parallel: one batch element per NeuronCore (8 cores, no collectives).

Per core, with xr = x[b] viewed as [C, S] (C=512 channels, S=1024 tokens):
    QT = wq^T @ xr + bq   -> [D, S]  (d on partitions; head h = rows 64h..64h+63)
    KT = wk^T @ xr + bk   -> [D, S]
    V  = xr^T @ wv + bv   -> [S, D]  (tokens on partitions)
    per head h: ET[j, i] = KT_h^T . QT_h            (keys j on psum partitions)
                E = exp(ET / sqrt(C))               (no max-subtract; |args| < ~3)
                O'T[d, i] = sum_j V[j, d] E[j, i];  Z[i] = sum_j E[j, i]
                OT[d, i] = O'T[d, i] / Z[i]
    y = wo^T @ OT + bo + xr   -> [C, S]

All matmul operands are fp16 (cast host-side; DMA moves 2B/elem). PSUM stays
fp32. Heads run in pairs (2t, 2t+1): the pair's K rows live in SBUF partitions
0..63 / 64..127, so the two K=64 energy matmuls run concurrently in disjoint
PE row groups.

V' stationary layout per (jc, h): 128 columns = [ones(64) | V(64)]. The AV
matmul therefore lands Z = sum_j E[j, i] REPLICATED on psum partitions 0..63
(and O' on 64..127) for every head, so the softmax normalization is just
  rb[0:64, h] = 1/ps_av[0:64, h]   (DVE reciprocal, 64 lanes)
  OT[64h..]   = ps_av[64:128, h] * rb[0:64, h]
with no cross-partition broadcast (the old gpsimd partition_broadcast is gone).

exp is split between ScalarE (exact, table-based) and the Vector engine
(Schraudolph bit-trick: exp(s*x) ~= bitcast_f16(u16(x*A + B)), one full-rate
tensor_scalar op; |rel err| < ~2%, which drowns in the softmax average).
ATTN_DVE_EXP selects how many of the 8 key-chunk units per (t, i) go to DVE.

Pipelining: energy pair -> exp -> (AV deferred one unit so the in-order PE
stream never waits on this unit's exp). PSUM (8 banks): energy units
double-buffered (4), one AV accumulator (2), QK/vproj/warm on a separate aux
buffer (2) so the hoisted next-pair QK matmuls never displace the energy
rotation. The previous half's norm is flushed to the DVE queue at each half
start (required: with a single AV buffer, a DVE-exp op queued ahead of the
norm would deadlock the PE FIFO). QK for head-pair t+1 is emitted mid-way
through t's stream to fill PE slack; the final projection's cc=3 accumulator
(the one gated on the last norm) is emitted after 18 cover matmuls.
"""

import math
import os

import numpy as np

B = 8
C = 512
S = 1024  # 32*32 tokens
NH = 8
HD = 64
P = 128
CC = C // P  # 4 contraction chunks of 128
NI = 2  # S split into 2 chunks of 512 for matmul free dim
SC = S // P  # 8 key chunks of 128

DVE_EXP = int(os.environ.get("ATTN_DVE_EXP", "1"))  # of 8 units per (t, i)
DIV_NORM = os.environ.get("ATTN_DIV_NORM", "0") == "1"
SCHRAUD_C = float(os.environ.get("ATTN_SCHRAUD_C", "44.75"))
N_WARM = int(os.environ.get("ATTN_WARM", "28"))

# Which key-chunk units of each (t, i) run exp on DVE. Positioned mid-half,
# right after the previous half's norm chain (2.7us) drains from the DVE
# FIFO — an exp-DVE op queued behind other DVE work stalls the 2-deep
# energy-tile rotation and starves ScalarE.
_DVE_UNITS = {0: (), 1: (4,), 2: (4, 5), 3: (3, 4, 5), 4: (3, 4, 5, 6)}.get(
    DVE_EXP, tuple(range(DVE_EXP))
)


def _emit(nc, tc, mybir, aps):
    import contextlib

    F32 = mybir.dt.float32
    F16 = mybir.dt.float16
    U16 = mybir.dt.uint16
    MULT = mybir.AluOpType.mult
    ADD = mybir.AluOpType.add
    DIV = mybir.AluOpType.divide
    EXP = mybir.ActivationFunctionType.Exp
    softmax_scale = 1.0 / math.sqrt(C)
    ONE_F16 = 0x3C00

    # Schraudolph constants for f16 bit-pattern exp of (softmax_scale * x):
    # u16(x * scale * 1024/ln2 + (15*1024 - c))
    SCH_A = softmax_scale * 1024.0 / math.log(2.0)
    SCH_B = 15.0 * 1024.0 - SCHRAUD_C

    xb, wq, bq, wk, bk, wv, bv, wo, bo, y = (
        aps[k] for k in ("xb", "wq", "bq", "wk", "bk", "wv", "bv", "wo", "bo", "y")
    )
    xb_r = xb.rearrange("(cc p) s -> p cc s", p=P)
    y_r = y.rearrange("(cc p) s -> p cc s", p=P)
    wq_r = wq.rearrange("(cc p) d -> p cc d", p=P)
    wk_r = wk.rearrange("(cc p) d -> p cc d", p=P)
    wv_r = wv.rearrange("(cc p) d -> p cc d", p=P)
    wo_r = wo.rearrange("(dc p) c -> p dc c", p=P)
    bq_r = bq.rearrange("(dc p) -> p dc", p=P)
    bk_r = bk.rearrange("(dc p) -> p dc", p=P)
    bo_r = bo.rearrange("(cc p) -> p cc", p=P)

    with contextlib.ExitStack() as ctx:
        singles = ctx.enter_context(tc.tile_pool(name="singles", bufs=1))
        qkpool = ctx.enter_context(tc.tile_pool(name="qk", bufs=2))
        etpool = ctx.enter_context(tc.tile_pool(name="et", bufs=8))
        rbpool = ctx.enter_context(tc.tile_pool(name="rb", bufs=4))
        tmppool = ctx.enter_context(tc.tile_pool(name="tmp", bufs=4))
        # PSUM bank budget (8 banks): energy 2 tiles x 2 banks, AV 1 x 2,
        # QK/vproj/warm (aux) 1 x 2. Keeping aux OUT of the energy pool means
        # the hoisted QK matmuls never displace the energy-tile rotation.
        pse = ctx.enter_context(tc.tile_pool(name="pse", bufs=2, space="PSUM"))
        psav = ctx.enter_context(tc.tile_pool(name="psav", bufs=1, space="PSUM"))
        psaux = ctx.enter_context(tc.tile_pool(name="psaux", bufs=1, space="PSUM"))

        # ---- input DMAs, spread across queues, first-needed first ----
        # sync: xb query-half 0; vector: xb half 1 (DVE idle at start)
        # scalar: bq/bk + wq/wk (t=0 columns first); gpsimd: bv/wv + bo/wo
        xb_sb = singles.tile([P, CC, S], F16)
        bq_sb = singles.tile([P, CC], F32)
        bk_sb = singles.tile([P, CC], F32)
        bo_sb = singles.tile([P, CC], F32)
        bv_sb = singles.tile([1, C], F16)
        wq_sb = singles.tile([P, CC, C], F16)
        wk_sb = singles.tile([P, CC, C], F16)
        wv_sb = singles.tile([P, CC, C], F16)
        wo_sb = singles.tile([P, CC, C], F16)

        # xb per-cc chunks (2KB contiguous rows) in the order the t=0 QK
        # accumulation consumes them; wq/wk per-cc interleaved likewise.
        for cc in range(CC):
            nc.sync.dma_start(out=xb_sb[:, cc], in_=xb_r[:, cc])
        nc.scalar.dma_start(out=wq_sb[:, 0], in_=wq_r[:, 0])
        nc.scalar.dma_start(out=wk_sb[:, 0], in_=wk_r[:, 0])
        nc.scalar.dma_start(out=bq_sb, in_=bq_r)
        nc.scalar.dma_start(out=bk_sb, in_=bk_r)
        for cc in range(1, CC):
            nc.scalar.dma_start(out=wq_sb[:, cc], in_=wq_r[:, cc])
            nc.scalar.dma_start(out=wk_sb[:, cc], in_=wk_r[:, cc])
        nc.gpsimd.dma_start(out=bv_sb, in_=bv[None, :])
        nc.gpsimd.dma_start(out=wv_sb, in_=wv_r)
        nc.gpsimd.dma_start(out=bo_sb, in_=bo_r)
        nc.gpsimd.dma_start(out=wo_sb, in_=wo_r)

        ones_row = singles.tile([1, P], F16)
        nc.vector.memset(ones_row.bitcast(U16), ONE_F16)

        # V' layout: 128 columns per (jc, h): [ones(64) | V(64)]. The AV matmul
        # (M=128) replicates Z = sum_j E[j, i] on psum rows 0..63 and puts O'
        # on rows 64..127 for every head. The ones halves are memset per key
        # chunk on GpSimd (idle engine; keeps the DVE queue clear for the t=0
        # bias adds); the V copies overwrite cols 64..127.
        Vp = singles.tile([P, SC, NH, P], F16)
        for sc in range(SC):
            nc.gpsimd.memset(Vp[:, sc, :, 0:64].bitcast(U16), ONE_F16)

        # PE warm-up on zeros while input DMAs land (HAM clock-gate at 8/8
        # before real matmuls start)
        warm = singles.tile([P, 512], F16)
        nc.vector.memset(warm.bitcast(U16), 0)
        ps_w = psaux.tile([P, 2, 512], F32, tag="aux")
        for _ in range(N_WARM):
            nc.tensor.matmul(ps_w[:, 0], warm[:, 0:128], warm)

        OTs = [singles.tile([P, S], F16, tag=f"ot{t}", name=f"ot{t}") for t in range(CC)]

        def emit_v_projection_chunk(sc):
            # V[s, d] = xr^T @ wv + bv for one token chunk
            ps_v = psaux.tile([P, 2, 512], F32, tag="aux")
            for cc in range(CC):
                nc.tensor.matmul(
                    ps_v[:, 0],
                    xb_sb[:, cc, sc * P : (sc + 1) * P],
                    wv_sb[:, cc],
                    start=(cc == 0),
                    stop=False,
                )
            nc.tensor.matmul(ps_v[:, 0], ones_row, bv_sb, start=False, stop=True)
            psv_r = ps_v[:, 0].rearrange("p (h d) -> p h d", h=NH)
            nc.vector.tensor_copy(out=Vp[:, sc, :, 64:128], in_=psv_r)

        pending_norm = [None]
        pending_av = []

        def flush_av(depth=0):
            while len(pending_av) > depth:
                pending_av.pop(0)()

        def flush_norm():
            if pending_norm[0] is not None:
                pending_norm[0]()
                pending_norm[0] = None

        qk_tiles = {}

        def emit_qk(t):
            # QT/KT for heads (2t, 2t+1)
            qt = qkpool.tile([P, S], F16, tag="qt", name=f"qt{t}")
            kt = qkpool.tile([P, S], F16, tag="kt", name=f"kt{t}")
            qk_tiles[t] = (qt, kt)
            for i in range(NI):
                sl = slice(i * 512, (i + 1) * 512)
                ps_p = psaux.tile([P, 2, 512], F32, tag="aux")
                for cc in range(CC):
                    xsl = xb_sb[:, cc, sl]
                    nc.tensor.matmul(
                        ps_p[:, 0],
                        wq_sb[:, cc, t * P : (t + 1) * P],
                        xsl,
                        start=(cc == 0),
                        stop=(cc == CC - 1),
                    )
                    nc.tensor.matmul(
                        ps_p[:, 1],
                        wk_sb[:, cc, t * P : (t + 1) * P],
                        xsl,
                        start=(cc == 0),
                        stop=(cc == CC - 1),
                    )
                nc.vector.tensor_scalar_add(qt[:, sl], ps_p[:, 0], bq_sb[:, t : t + 1])
                nc.vector.tensor_scalar_add(kt[:, sl], ps_p[:, 1], bk_sb[:, t : t + 1])

        # ---- per head-pair t ----
        # QK for t+1 is emitted mid-way through t's first query-half, where it
        # fills the PE slack of the ScalarE-bound exp stream (instead of
        # stalling ACT for ~4.7us at every head-pair seam).
        emit_qk(0)
        for t in range(CC):
            qt, kt = qk_tiles.pop(t)
            # energy -> exp -> AV, pipelined per (query-half i, key-chunk jc).
            h0, h1 = 2 * t, 2 * t + 1
            for i in range(NI):
                sl = slice(i * 512, (i + 1) * 512)
                # The previous half's norm must be emitted to the DVE queue
                # before this half's DVE-exp units: with a single AV buffer,
                # an exp-DVE op queued ahead of the norm would deadlock the
                # PE FIFO (av waits buffer <- norm waits exp <- energy behind
                # the stalled av).
                flush_norm()
                ps_av = psav.tile([P, 2, 512], F32, tag="av")  # h0, h1
                for jc in range(SC):
                    if t == 0 and i == 0:
                        emit_v_projection_chunk(jc)
                    hoist = (1, 6) if t == 0 else (0, 6)
                    if (i, jc) == hoist and t < CC - 1:
                        emit_qk(t + 1)
                    k0 = kt[0:64, jc * P : (jc + 1) * P]
                    k1 = kt[64:128, jc * P : (jc + 1) * P]
                    first, last = jc == 0, jc == SC - 1
                    ps_e = pse.tile([P, 2, 512], F32, tag="e")  # head-major
                    nc.tensor.matmul(ps_e[:, 0], k0, qt[0:64, sl])
                    nc.tensor.matmul(ps_e[:, 1], k1, qt[64:128, sl])
                    et = etpool.tile([P, 2, 512], F16, tag="et")
                    if jc in _DVE_UNITS:
                        # Schraudolph f16 bit-pattern exp on DVE
                        nc.vector.tensor_scalar(
                            et.bitcast(U16), ps_e, SCH_A, SCH_B, MULT, ADD
                        )
                    else:
                        nc.scalar.activation(
                            out=et, in_=ps_e, func=EXP, scale=softmax_scale
                        )
                    # AV emitted one unit late so the in-order PE stream never
                    # waits on this unit's exp
                    flush_av(depth=1)

                    def av(ps_av=ps_av, jc=jc, et=et, h0=h0, h1=h1,
                           first=first, last=last):
                        nc.tensor.matmul(
                            ps_av[:, 0], Vp[:, jc, h0], et[:, 0],
                            start=first, stop=last,
                        )
                        nc.tensor.matmul(
                            ps_av[:, 1], Vp[:, jc, h1], et[:, 1],
                            start=first, stop=last,
                        )

                    pending_av.append(av)

                flush_av()

                def norm(t=t, sl=sl, ps_av=ps_av):
                    # Z replicated on psum rows 0..63 (ones half of V'); O' on
                    # rows 64..127. Reciprocal + multiply read partition-
                    # aligned rows; builtin DVE ops may cross partition bases.
                    if DIV_NORM:
                        nc.vector.tensor_tensor(
                            OTs[t][0:64, sl], ps_av[64:128, 0], ps_av[0:64, 0], DIV
                        )
                        nc.vector.tensor_tensor(
                            OTs[t][64:128, sl], ps_av[64:128, 1], ps_av[0:64, 1], DIV
                        )
                    else:
                        rb = rbpool.tile([64, 2, 512], F32, tag="rb")
                        nc.vector.reciprocal_approx_fast(out=rb, in_=ps_av[0:64])
                        nc.vector.tensor_tensor(
                            OTs[t][0:64, sl], ps_av[64:128, 0], rb[:, 0], MULT
                        )
                        nc.vector.tensor_tensor(
                            OTs[t][64:128, sl], ps_av[64:128, 1], rb[:, 1], MULT
                        )

                pending_norm[0] = norm

        # ---- final projection + bias + residual ----
        # All four cc accumulators live at once (two from each drained pool),
        # so the 32 matmuls run back-to-back; dc=3 waits only on the deferred
        # t=3 normalization, which overlaps dc=0..2.
        # cc=0..2 accumulators have no norm dependency (their psum buffers are
        # freed by exp / the early norm); cc=3's buffer waits on the deferred
        # t=3 i=1 norm, so its matmuls are emitted after 18 cover matmuls.
        flush_norm()
        ps_fs = [
            pse.tile([P, 2, 512], F32, tag="e", name="psf0"),
            pse.tile([P, 2, 512], F32, tag="e", name="psf1"),
            psaux.tile([P, 2, 512], F32, tag="aux", name="psf2"),
            psav.tile([P, 2, 512], F32, tag="av", name="psf3"),
        ]

        def fmm(dc, cc, start, stop):
            wo_sl = wo_sb[:, dc, cc * P : (cc + 1) * P]
            for i in range(NI):
                sl = slice(i * 512, (i + 1) * 512)
                nc.tensor.matmul(
                    ps_fs[cc][:, i], wo_sl, OTs[dc][:, sl], start=start, stop=stop,
                )

        for dc in range(CC - 1):
            for cc in range(CC - 1):
                fmm(dc, cc, dc == 0, False)
        for dc in range(CC - 1):
            fmm(dc, 3, dc == 0, False)
        out_q = [nc.sync, nc.scalar, nc.gpsimd]
        for cc in range(CC):
            fmm(CC - 1, cc, False, True)
            # per-half STT + store so DVE/DMA pipeline with the cc loop
            for i in range(NI):
                sl = slice(i * 512, (i + 1) * 512)
                tmp = tmppool.tile([P, 512], F16, tag="tmp")
                nc.vector.scalar_tensor_tensor(
                    out=tmp,
                    in0=ps_fs[cc][:, i],
                    scalar=bo_sb[:, cc : cc + 1],
                    in1=xb_sb[:, cc, sl],
                    op0=ADD,
                    op1=ADD,
                )
                out_q[(2 * cc + i) % 3].dma_start(out=y_r[:, cc, sl], in_=tmp)


_NC_CACHE = {}


def _build():
    key = (DVE_EXP, DIV_NORM, SCHRAUD_C, N_WARM)
    if key in _NC_CACHE:
        return _NC_CACHE[key]
    import concourse.bacc as bacc
    import concourse.mybir as mybir
    import concourse.tile as tile

    F32 = mybir.dt.float32
    F16 = mybir.dt.float16
    nc = bacc.Bacc("TRN2", target_bir_lowering=False, debug=False)
    aps = {}
    aps["xb"] = nc.dram_tensor("xb", (C, S), F16, kind="ExternalInput").ap()
    for name in ("wq", "wk", "wv", "wo"):
        aps[name] = nc.dram_tensor(name, (C, C), F16, kind="ExternalInput").ap()
    for name in ("bq", "bk", "bo"):
        aps[name] = nc.dram_tensor(name, (C,), F32, kind="ExternalInput").ap()
    aps["bv"] = nc.dram_tensor("bv", (C,), F16, kind="ExternalInput").ap()
    # y in f16: halves the output DMA tail; the extra ~5e-4 rounding is far
    # inside the tolerance. The host upcasts back to f32.
    aps["y"] = nc.dram_tensor("y", (C, S), F16, kind="ExternalOutput").ap()
    with tile.TileContext(nc) as tc:
        _emit(nc, tc, mybir, aps)
    nc.compile()
    _NC_CACHE[key] = nc
    return nc


def prepare_in_maps(x, wq, bq, wk, bk, wv, bv, wo, bo):
    """Host-side prep: cast activations/weights to f16, shard x per core."""
    x = np.asarray(x, dtype=np.float32).reshape(B, C, S)
    weights = {
        "wq": np.ascontiguousarray(np.asarray(wq, dtype=np.float16)),
        "bq": np.ascontiguousarray(np.asarray(bq, dtype=np.float32)),
        "wk": np.ascontiguousarray(np.asarray(wk, dtype=np.float16)),
        "bk": np.ascontiguousarray(np.asarray(bk, dtype=np.float32)),
        "wv": np.ascontiguousarray(np.asarray(wv, dtype=np.float16)),
        "bv": np.ascontiguousarray(np.asarray(bv, dtype=np.float16)),
        "wo": np.ascontiguousarray(np.asarray(wo, dtype=np.float16)),
        "bo": np.ascontiguousarray(np.asarray(bo, dtype=np.float32)),
    }
    return [
        {"xb": np.ascontiguousarray(x[b].astype(np.float16)), **weights}
        for b in range(B)
    ]


def kernel(x, wq, bq, wk, bk, wv, bv, wo, bo):
    from concourse import bass_utils

    nc = _build()
    in_maps = prepare_in_maps(x, wq, bq, wk, bk, wv, bv, wo, bo)
    res = bass_utils.run_bass_kernel_spmd(nc, in_maps, core_ids=list(range(B)))
    out = np.stack([r["y"].astype(np.float32) for r in res.results])
    return out.reshape(B, C, 32, 32)


# revision 35
# speedup vs baseline: 1.0435x; 1.0173x over previous
"""AttentionBlock Trainium2 kernel (v2).

Data-parallel: one batch element per NeuronCore (8 cores, no collectives).

Per core, with xr = x[b] viewed as [C, S] (C=512 channels, S=1024 tokens):
    QT = wq^T @ xr + bq   -> [D, S]  (d on partitions; head h = rows 64h..64h+63)
    KT = wk^T @ xr + bk   -> [D, S]
    V  = xr^T @ wv + bv   -> [S, D]  (tokens on partitions)
    per head h: ET[j, i] = KT_h^T . QT_h            (keys j on psum partitions)
                E = exp(ET / sqrt(C))               (no max-subtract; |args| < ~3)
                O'T[d, i] = sum_j V[j, d] E[j, i];  Z[i] = sum_j E[j, i]
                OT[d, i] = O'T[d, i] / Z[i]
    y = wo^T @ OT + bo + xr   -> [C, S]

All matmul operands are fp16 (cast host-side; DMA moves 2B/elem). PSUM stays
fp32. Heads run in pairs (2t, 2t+1): the pair's K rows live in SBUF partitions
0..63 / 64..127, so the two K=64 energy matmuls run concurrently in disjoint
PE row groups.

V' stationary layout per (jc, h): 128 columns = [ones(64) | V(64)]. The AV
matmul therefore lands Z = sum_j E[j, i] REPLICATED on psum partitions 0..63
(and O' on 64..127) for every head, so the softmax normalization is just
  rb[0:64, h] = 1/ps_av[0:64, h]   (DVE reciprocal, 64 lanes)
  OT[64h..]   = ps_av[64:128, h] * rb[0:64, h]
with no cross-partition broadcast (the old gpsimd partition_broadcast is gone).

exp is split between ScalarE (exact, table-based) and the Vector engine
(Schraudolph bit-trick: exp(s*x) ~= bitcast_f16(u16(x*A + B)), one full-rate
tensor_scalar op; |rel err| < ~2%, which drowns in the softmax average).
ATTN_DVE_EXP selects how many of the 8 key-chunk units per (t, i) go to DVE.

Pipelining: energy pair -> exp -> (AV deferred one unit so the in-order PE
stream never waits on this unit's exp). PSUM (8 banks): energy units
double-buffered (4), one AV accumulator (2), QK/vproj/warm on a separate aux
buffer (2) so the hoisted next-pair QK matmuls never displace the energy
rotation. The previous half's norm is flushed to the DVE queue at each half
start (required: with a single AV buffer, a DVE-exp op queued ahead of the
norm would deadlock the PE FIFO). QK for head-pair t+1 is emitted mid-way
through t's stream to fill PE slack; the final projection's cc=3 accumulator
(the one gated on the last norm) is emitted after 18 cover matmuls.
"""

import math
import os

import numpy as np

B = 8
C = 512
S = 1024  # 32*32 tokens
NH = 8
HD = 64
P = 128
CC = C // P  # 4 contraction chunks of 128
NI = 2  # S split into 2 chunks of 512 for matmul free dim
SC = S // P  # 8 key chunks of 128

DVE_EXP = int(os.environ.get("ATTN_DVE_EXP", "1"))  # of 8 units per (t, i)
DIV_NORM = os.environ.get("ATTN_DIV_NORM", "0") == "1"
SCHRAUD_C = float(os.environ.get("ATTN_SCHRAUD_C", "44.75"))
N_WARM = int(os.environ.get("ATTN_WARM", "28"))

# Which key-chunk units of each (t, i) run exp on DVE. Positioned mid-half,
# right after the previous half's norm chain (2.7us) drains from the DVE
# FIFO — an exp-DVE op queued behind other DVE work stalls the 2-deep
# energy-tile rotation and starves ScalarE.
_DVE_UNITS = {0: (), 1: (4,), 2: (4, 5), 3: (3, 4, 5), 4: (3, 4, 5, 6)}.get(
    DVE_EXP, tuple(range(DVE_EXP))
)


def _emit(nc, tc, mybir, aps):
    import contextlib

    F32 = mybir.dt.float32
    F16 = mybir.dt.float16
    U16 = mybir.dt.uint16
    MULT = mybir.AluOpType.mult
    ADD = mybir.AluOpType.add
    DIV = mybir.AluOpType.divide
    EXP = mybir.ActivationFunctionType.Exp
    softmax_scale = 1.0 / math.sqrt(C)
    ONE_F16 = 0x3C00

    # Schraudolph constants for f16 bit-pattern exp of (softmax_scale * x):
    # u16(x * scale * 1024/ln2 + (15*1024 - c))
    SCH_A = softmax_scale * 1024.0 / math.log(2.0)
    SCH_B = 15.0 * 1024.0 - SCHRAUD_C

    xb, wq, bq, wk, bk, wv, bv, wo, bo, y = (
        aps[k] for k in ("xb", "wq", "bq", "wk", "bk", "wv", "bv", "wo", "bo", "y")
    )
    xb_r = xb.rearrange("(cc p) s -> p cc s", p=P)
    y_r = y.rearrange("(cc p) s -> p cc s", p=P)
    wq_r = wq.rearrange("(cc p) d -> p cc d", p=P)
    wk_r = wk.rearrange("(cc p) d -> p cc d", p=P)
    wv_r = wv.rearrange("(cc p) d -> p cc d", p=P)
    wo_r = wo.rearrange("(dc p) c -> p dc c", p=P)
    bq_r = bq.rearrange("(dc p) -> p dc", p=P)
    bk_r = bk.rearrange("(dc p) -> p dc", p=P)
    bo_r = bo.rearrange("(cc p) -> p cc", p=P)

    with contextlib.ExitStack() as ctx:
        singles = ctx.enter_context(tc.tile_pool(name="singles", bufs=1))
        qkpool = ctx.enter_context(tc.tile_pool(name="qk", bufs=2))
        etpool = ctx.enter_context(tc.tile_pool(name="et", bufs=8))
        rbpool = ctx.enter_context(tc.tile_pool(name="rb", bufs=4))
        tmppool = ctx.enter_context(tc.tile_pool(name="tmp", bufs=4))
        # PSUM bank budget (8 banks): energy 2 tiles x 2 banks, AV 1 x 2,
        # QK/vproj/warm (aux) 1 x 2. Keeping aux OUT of the energy pool means
        # the hoisted QK matmuls never displace the energy-tile rotation.
        pse = ctx.enter_context(tc.tile_pool(name="pse", bufs=2, space="PSUM"))
        psav = ctx.enter_context(tc.tile_pool(name="psav", bufs=1, space="PSUM"))
        psaux = ctx.enter_context(tc.tile_pool(name="psaux", bufs=1, space="PSUM"))

        # ---- input DMAs, spread across queues, first-needed first ----
        # sync: xb query-half 0; vector: xb half 1 (DVE idle at start)
        # scalar: bq/bk + wq/wk (t=0 columns first); gpsimd: bv/wv + bo/wo
        xb_sb = singles.tile([P, CC, S], F16)
        bq_sb = singles.tile([P, CC], F32)
        bk_sb = singles.tile([P, CC], F32)
        bo_sb = singles.tile([P, CC], F32)
        bv_sb = singles.tile([1, C], F16)
        wq_sb = singles.tile([P, CC, C], F16)
        wk_sb = singles.tile([P, CC, C], F16)
        wv_sb = singles.tile([P, CC, C], F16)
        wo_sb = singles.tile([P, CC, C], F16)

        # xb per-cc chunks (2KB contiguous rows) in the order the t=0 QK
        # accumulation consumes them; wq/wk per-cc interleaved likewise.
        for cc in range(CC):
            nc.sync.dma_start(out=xb_sb[:, cc], in_=xb_r[:, cc])
        nc.scalar.dma_start(out=wq_sb[:, 0], in_=wq_r[:, 0])
        nc.scalar.dma_start(out=wk_sb[:, 0], in_=wk_r[:, 0])
        nc.scalar.dma_start(out=bq_sb, in_=bq_r)
        nc.scalar.dma_start(out=bk_sb, in_=bk_r)
        for cc in range(1, CC):
            nc.scalar.dma_start(out=wq_sb[:, cc], in_=wq_r[:, cc])
            nc.scalar.dma_start(out=wk_sb[:, cc], in_=wk_r[:, cc])
        nc.gpsimd.dma_start(out=bv_sb, in_=bv[None, :])
        nc.gpsimd.dma_start(out=wv_sb, in_=wv_r)
        nc.gpsimd.dma_start(out=bo_sb, in_=bo_r)
        nc.gpsimd.dma_start(out=wo_sb, in_=wo_r)

        ones_row = singles.tile([1, P], F16)
        nc.vector.memset(ones_row.bitcast(U16), ONE_F16)

        # V' layout: 128 columns per (jc, h): [ones(64) | V(64)]. The AV matmul
        # (M=128) replicates Z = sum_j E[j, i] on psum rows 0..63 and puts O'
        # on rows 64..127 for every head. The ones halves are memset per key
        # chunk on GpSimd (idle engine; keeps the DVE queue clear for the t=0
        # bias adds); the V copies overwrite cols 64..127.
        Vp = singles.tile([P, SC, NH, P], F16)
        for sc in range(SC):
            nc.gpsimd.memset(Vp[:, sc, :, 0:64].bitcast(U16), ONE_F16)

        # PE warm-up on zeros while input DMAs land (HAM clock-gate at 8/8
        # before real matmuls start)
        warm = singles.tile([P, 512], F16)
        nc.vector.memset(warm.bitcast(U16), 0)
        ps_w = psaux.tile([P, 2, 512], F32, tag="aux")
        for _ in range(N_WARM):
            nc.tensor.matmul(ps_w[:, 0], warm[:, 0:128], warm)

        OTs = [singles.tile([P, S], F16, tag=f"ot{t}", name=f"ot{t}") for t in range(CC)]

        def emit_v_projection_chunk(sc):
            # V[s, d] = xr^T @ wv + bv for one token chunk
            ps_v = pse.tile([P, 2, 512], F32, tag="e")
            for cc in range(CC):
                nc.tensor.matmul(
                    ps_v[:, 0],
                    xb_sb[:, cc, sc * P : (sc + 1) * P],
                    wv_sb[:, cc],
                    start=(cc == 0),
                    stop=False,
                )
            nc.tensor.matmul(ps_v[:, 0], ones_row, bv_sb, start=False, stop=True)
            psv_r = ps_v[:, 0].rearrange("p (h d) -> p h d", h=NH)
            nc.vector.tensor_copy(out=Vp[:, sc, :, 64:128], in_=psv_r)

        pending_norm = [None]
        pending_av = []

        def flush_av(depth=0):
            while len(pending_av) > depth:
                pending_av.pop(0)()

        def flush_norm():
            if pending_norm[0] is not None:
                pending_norm[0]()
                pending_norm[0] = None

        qk_tiles = {}

        def make_qk_steps(tn):
            # Hoisted QK for head-pair tn, split in three: i0 matmuls; i0
            # bias + i1 matmuls; i1 bias. Spreading them keeps the 16-matmul
            # block off the head-pair seam (where it starved ScalarE for
            # ~5us) and keeps the DVE bias ops away from the DVE-exp unit.
            qt = qkpool.tile([P, S], F16, tag="qt", name=f"qt{tn}")
            kt = qkpool.tile([P, S], F16, tag="kt", name=f"kt{tn}")
            qk_tiles[tn] = (qt, kt)
            ps_list = []

            def mms(i):
                sl = slice(i * 512, (i + 1) * 512)
                ps_p = psaux.tile([P, 2, 512], F32, tag="aux")
                ps_list.append(ps_p)
                for cc in range(CC):
                    xsl = xb_sb[:, cc, sl]
                    nc.tensor.matmul(
                        ps_p[:, 0], wq_sb[:, cc, tn * P : (tn + 1) * P], xsl,
                        start=(cc == 0), stop=(cc == CC - 1),
                    )
                    nc.tensor.matmul(
                        ps_p[:, 1], wk_sb[:, cc, tn * P : (tn + 1) * P], xsl,
                        start=(cc == 0), stop=(cc == CC - 1),
                    )

            def bias(i):
                sl = slice(i * 512, (i + 1) * 512)
                nc.vector.tensor_scalar_add(
                    qt[:, sl], ps_list[i][:, 0], bq_sb[:, tn : tn + 1]
                )
                nc.vector.tensor_scalar_add(
                    kt[:, sl], ps_list[i][:, 1], bk_sb[:, tn : tn + 1]
                )

            return [
                lambda: mms(0),
                lambda: (bias(0), mms(1)),
                lambda: bias(1),
            ]

        def emit_qk(t):
            # QT/KT for heads (2t, 2t+1)
            qt = qkpool.tile([P, S], F16, tag="qt", name=f"qt{t}")
            kt = qkpool.tile([P, S], F16, tag="kt", name=f"kt{t}")
            qk_tiles[t] = (qt, kt)
            for i in range(NI):
                sl = slice(i * 512, (i + 1) * 512)
                ps_p = psaux.tile([P, 2, 512], F32, tag="aux")
                for cc in range(CC):
                    xsl = xb_sb[:, cc, sl]
                    nc.tensor.matmul(
                        ps_p[:, 0],
                        wq_sb[:, cc, t * P : (t + 1) * P],
                        xsl,
                        start=(cc == 0),
                        stop=(cc == CC - 1),
                    )
                    nc.tensor.matmul(
                        ps_p[:, 1],
                        wk_sb[:, cc, t * P : (t + 1) * P],
                        xsl,
                        start=(cc == 0),
                        stop=(cc == CC - 1),
                    )
                nc.vector.tensor_scalar_add(qt[:, sl], ps_p[:, 0], bq_sb[:, t : t + 1])
                nc.vector.tensor_scalar_add(kt[:, sl], ps_p[:, 1], bk_sb[:, t : t + 1])

        # ---- per head-pair t ----
        # QK for t+1 is emitted mid-way through t's first query-half, where it
        # fills the PE slack of the ScalarE-bound exp stream (instead of
        # stalling ACT for ~4.7us at every head-pair seam).
        emit_qk(0)
        for t in range(CC):
            qt, kt = qk_tiles.pop(t)
            # energy -> exp -> AV, pipelined per (query-half i, key-chunk jc).
            h0, h1 = 2 * t, 2 * t + 1
            for i in range(NI):
                sl = slice(i * 512, (i + 1) * 512)
                # The previous half's norm must be emitted to the DVE queue
                # before this half's DVE-exp units: with a single AV buffer,
                # an exp-DVE op queued ahead of the norm would deadlock the
                # PE FIFO (av waits buffer <- norm waits exp <- energy behind
                # the stalled av).
                flush_norm()
                ps_av = psav.tile([P, 2, 512], F32, tag="av")  # h0, h1
                for jc in range(SC):
                    if t == 0 and i == 0:
                        emit_v_projection_chunk(jc)
                    ih = 1 if t == 0 else 0
                    if i == ih and t < CC - 1:
                        if jc == 2:
                            qk_steps = make_qk_steps(t + 1)
                            qk_steps[0]()
                        elif jc == 5:
                            qk_steps[1]()
                        elif jc == 7:
                            qk_steps[2]()
                    k0 = kt[0:64, jc * P : (jc + 1) * P]
                    k1 = kt[64:128, jc * P : (jc + 1) * P]
                    first, last = jc == 0, jc == SC - 1
                    ps_e = pse.tile([P, 2, 512], F32, tag="e")  # head-major
                    nc.tensor.matmul(ps_e[:, 0], k0, qt[0:64, sl])
                    nc.tensor.matmul(ps_e[:, 1], k1, qt[64:128, sl])
                    et = etpool.tile([P, 2, 512], F16, tag="et")
                    if jc in _DVE_UNITS:
                        # Schraudolph f16 bit-pattern exp on DVE
                        nc.vector.tensor_scalar(
                            et.bitcast(U16), ps_e, SCH_A, SCH_B, MULT, ADD
                        )
                    else:
                        nc.scalar.activation(
                            out=et, in_=ps_e, func=EXP, scale=softmax_scale
                        )
                    # AV emitted one unit late so the in-order PE stream never
                    # waits on this unit's exp
                    flush_av(depth=1)

                    def av(ps_av=ps_av, jc=jc, et=et, h0=h0, h1=h1,
                           first=first, last=last):
                        nc.tensor.matmul(
                            ps_av[:, 0], Vp[:, jc, h0], et[:, 0],
                            start=first, stop=last,
                        )
                        nc.tensor.matmul(
                            ps_av[:, 1], Vp[:, jc, h1], et[:, 1],
                            start=first, stop=last,
                        )

                    pending_av.append(av)

                flush_av()

                def norm(t=t, sl=sl, ps_av=ps_av):
                    # Z replicated on psum rows 0..63 (ones half of V'); O' on
                    # rows 64..127. Reciprocal + multiply read partition-
                    # aligned rows; builtin DVE ops may cross partition bases.
                    if DIV_NORM:
                        nc.vector.tensor_tensor(
                            OTs[t][0:64, sl], ps_av[64:128, 0], ps_av[0:64, 0], DIV
                        )
                        nc.vector.tensor_tensor(
                            OTs[t][64:128, sl], ps_av[64:128, 1], ps_av[0:64, 1], DIV
                        )
                    else:
                        rb = rbpool.tile([64, 2, 512], F32, tag="rb")
                        nc.vector.reciprocal_approx_fast(out=rb, in_=ps_av[0:64])
                        nc.vector.tensor_tensor(
                            OTs[t][0:64, sl], ps_av[64:128, 0], rb[:, 0], MULT
                        )
                        nc.vector.tensor_tensor(
                            OTs[t][64:128, sl], ps_av[64:128, 1], rb[:, 1], MULT
                        )

                pending_norm[0] = norm

        # ---- final projection + bias + residual ----
        # All four cc accumulators live at once (two from each drained pool),
        # so the 32 matmuls run back-to-back; dc=3 waits only on the deferred
        # t=3 normalization, which overlaps dc=0..2.
        # cc=0..2 accumulators have no norm dependency (their psum buffers are
        # freed by exp / the early norm); cc=3's buffer waits on the deferred
        # t=3 i=1 norm, so its matmuls are emitted after 18 cover matmuls.
        flush_norm()
        ps_fs = [
            pse.tile([P, 2, 512], F32, tag="e", name="psf0"),
            pse.tile([P, 2, 512], F32, tag="e", name="psf1"),
            psaux.tile([P, 2, 512], F32, tag="aux", name="psf2"),
            psav.tile([P, 2, 512], F32, tag="av", name="psf3"),
        ]

        def fmm(dc, cc, start, stop):
            wo_sl = wo_sb[:, dc, cc * P : (cc + 1) * P]
            for i in range(NI):
                sl = slice(i * 512, (i + 1) * 512)
                nc.tensor.matmul(
                    ps_fs[cc][:, i], wo_sl, OTs[dc][:, sl], start=start, stop=stop,
                )

        for dc in range(CC - 1):
            for cc in range(CC - 1):
                fmm(dc, cc, dc == 0, False)
        for dc in range(CC - 1):
            fmm(dc, 3, dc == 0, False)
        out_q = [nc.sync, nc.scalar, nc.gpsimd]
        for cc in range(CC):
            fmm(CC - 1, cc, False, True)
            # per-half STT + store so DVE/DMA pipeline with the cc loop
            for i in range(NI):
                sl = slice(i * 512, (i + 1) * 512)
                tmp = tmppool.tile([P, 512], F16, tag="tmp")
                nc.vector.scalar_tensor_tensor(
                    out=tmp,
                    in0=ps_fs[cc][:, i],
                    scalar=bo_sb[:, cc : cc + 1],
                    in1=xb_sb[:, cc, sl],
                    op0=ADD,
                    op1=ADD,
                )
                out_q[(2 * cc + i) % 3].dma_start(out=y_r[:, cc, sl], in_=tmp)


_NC_CACHE = {}


def _build():
    key = (DVE_EXP, DIV_NORM, SCHRAUD_C, N_WARM)
    if key in _NC_CACHE:
        return _NC_CACHE[key]
    import concourse.bacc as bacc
    import concourse.mybir as mybir
    import concourse.tile as tile

    F32 = mybir.dt.float32
    F16 = mybir.dt.float16
    nc = bacc.Bacc("TRN2", target_bir_lowering=False, debug=False)
    aps = {}
    aps["xb"] = nc.dram_tensor("xb", (C, S), F16, kind="ExternalInput").ap()
    for name in ("wq", "wk", "wv", "wo"):
        aps[name] = nc.dram_tensor(name, (C, C), F16, kind="ExternalInput").ap()
    for name in ("bq", "bk", "bo"):
        aps[name] = nc.dram_tensor(name, (C,), F32, kind="ExternalInput").ap()
    aps["bv"] = nc.dram_tensor("bv", (C,), F16, kind="ExternalInput").ap()
    # y in f16: halves the output DMA tail; the extra ~5e-4 rounding is far
    # inside the tolerance. The host upcasts back to f32.
    aps["y"] = nc.dram_tensor("y", (C, S), F16, kind="ExternalOutput").ap()
    with tile.TileContext(nc) as tc:
        _emit(nc, tc, mybir, aps)
    nc.compile()
    _NC_CACHE[key] = nc
    return nc


def prepare_in_maps(x, wq, bq, wk, bk, wv, bv, wo, bo):
    """Host-side prep: cast activations/weights to f16, shard x per core."""
    x = np.asarray(x, dtype=np.float32).reshape(B, C, S)
    weights = {
        "wq": np.ascontiguousarray(np.asarray(wq, dtype=np.float16)),
        "bq": np.ascontiguousarray(np.asarray(bq, dtype=np.float32)),
        "wk": np.ascontiguousarray(np.asarray(wk, dtype=np.float16)),
        "bk": np.ascontiguousarray(np.asarray(bk, dtype=np.float32)),
        "wv": np.ascontiguousarray(np.asarray(wv, dtype=np.float16)),
        "bv": np.ascontiguousarray(np.asarray(bv, dtype=np.float16)),
        "wo": np.ascontiguousarray(np.asarray(wo, dtype=np.float16)),
        "bo": np.ascontiguousarray(np.asarray(bo, dtype=np.float32)),
    }
    return [
        {"xb": np.ascontiguousarray(x[b].astype(np.float16)), **weights}
        for b in range(B)
    ]


def kernel(x, wq, bq, wk, bk, wv, bv, wo, bo):
    from concourse import bass_utils

    nc = _build()
    in_maps = prepare_in_maps(x, wq, bq, wk, bk, wv, bv, wo, bo)
    res = bass_utils.run_bass_kernel_spmd(nc, in_maps, core_ids=list(range(B)))
    out = np.stack([r["y"].astype(np.float32) for r in res.results])
    return out.reshape(B, C, 32, 32)


# revision 36
# speedup vs baseline: 1.0459x; 1.0023x over previous
"""AttentionBlock Trainium2 kernel (v2).

Data-parallel: one batch element per NeuronCore (8 cores, no collectives).

Per core, with xr = x[b] viewed as [C, S] (C=512 channels, S=1024 tokens):
    QT = wq^T @ xr + bq   -> [D, S]  (d on partitions; head h = rows 64h..64h+63)
    KT = wk^T @ xr + bk   -> [D, S]
    V  = xr^T @ wv + bv   -> [S, D]  (tokens on partitions)
    per head h: ET[j, i] = KT_h^T . QT_h            (keys j on psum partitions)
                E = exp(ET / sqrt(C))               (no max-subtract; |args| < ~3)
                O'T[d, i] = sum_j V[j, d] E[j, i];  Z[i] = sum_j E[j, i]
                OT[d, i] = O'T[d, i] / Z[i]
    y = wo^T @ OT + bo + xr   -> [C, S]

All matmul operands are fp16 (cast host-side; DMA moves 2B/elem). PSUM stays
fp32. Heads run in pairs (2t, 2t+1): the pair's K rows live in SBUF partitions
0..63 / 64..127, so the two K=64 energy matmuls run concurrently in disjoint
PE row groups.

V' stationary layout per (jc, h): 128 columns = [ones(64) | V(64)]. The AV
matmul therefore lands Z = sum_j E[j, i] REPLICATED on psum partitions 0..63
(and O' on 64..127) for every head, so the softmax normalization is just
  rb[0:64, h] = 1/ps_av[0:64, h]   (DVE reciprocal, 64 lanes)
  OT[64h..]   = ps_av[64:128, h] * rb[0:64, h]
with no cross-partition broadcast (the old gpsimd partition_broadcast is gone).

exp is split between ScalarE (exact, table-based) and the Vector engine
(Schraudolph bit-trick: exp(s*x) ~= bitcast_f16(u16(x*A + B)), one full-rate
tensor_scalar op; |rel err| < ~2%, which drowns in the softmax average).
ATTN_DVE_EXP selects how many of the 8 key-chunk units per (t, i) go to DVE.

Pipelining: energy pair -> exp -> (AV deferred one unit so the in-order PE
stream never waits on this unit's exp). PSUM (8 banks): energy units
double-buffered (4), one AV accumulator (2), QK/vproj/warm on a separate aux
buffer (2) so the hoisted next-pair QK matmuls never displace the energy
rotation. The previous half's norm is flushed to the DVE queue at each half
start (required: with a single AV buffer, a DVE-exp op queued ahead of the
norm would deadlock the PE FIFO). QK for head-pair t+1 is emitted mid-way
through t's stream to fill PE slack; the final projection's cc=3 accumulator
(the one gated on the last norm) is emitted after 18 cover matmuls.
"""

import math
import os

import numpy as np

B = 8
C = 512
S = 1024  # 32*32 tokens
NH = 8
HD = 64
P = 128
CC = C // P  # 4 contraction chunks of 128
NI = 2  # S split into 2 chunks of 512 for matmul free dim
SC = S // P  # 8 key chunks of 128

DVE_EXP = int(os.environ.get("ATTN_DVE_EXP", "1"))  # of 8 units per (t, i)
DIV_NORM = os.environ.get("ATTN_DIV_NORM", "0") == "1"
SCHRAUD_C = float(os.environ.get("ATTN_SCHRAUD_C", "44.75"))
N_WARM = int(os.environ.get("ATTN_WARM", "28"))

# Which key-chunk units of each (t, i) run exp on DVE. Positioned mid-half,
# right after the previous half's norm chain (2.7us) drains from the DVE
# FIFO — an exp-DVE op queued behind other DVE work stalls the 2-deep
# energy-tile rotation and starves ScalarE.
_DVE_UNITS = {0: (), 1: (4,), 2: (4, 5), 3: (3, 4, 5), 4: (3, 4, 5, 6)}.get(
    DVE_EXP, tuple(range(DVE_EXP))
)


def _emit(nc, tc, mybir, aps):
    import contextlib

    F32 = mybir.dt.float32
    F16 = mybir.dt.float16
    U16 = mybir.dt.uint16
    MULT = mybir.AluOpType.mult
    ADD = mybir.AluOpType.add
    DIV = mybir.AluOpType.divide
    EXP = mybir.ActivationFunctionType.Exp
    softmax_scale = 1.0 / math.sqrt(C)
    ONE_F16 = 0x3C00

    # Schraudolph constants for f16 bit-pattern exp of (softmax_scale * x):
    # u16(x * scale * 1024/ln2 + (15*1024 - c))
    SCH_A = softmax_scale * 1024.0 / math.log(2.0)
    SCH_B = 15.0 * 1024.0 - SCHRAUD_C

    xb, wq, bq, wk, bk, wv, bv, wo, bo, y = (
        aps[k] for k in ("xb", "wq", "bq", "wk", "bk", "wv", "bv", "wo", "bo", "y")
    )
    xb_r = xb.rearrange("(cc p) s -> p cc s", p=P)
    y_r = y.rearrange("(cc p) s -> p cc s", p=P)
    wq_r = wq.rearrange("(cc p) d -> p cc d", p=P)
    wk_r = wk.rearrange("(cc p) d -> p cc d", p=P)
    wv_r = wv.rearrange("(cc p) d -> p cc d", p=P)
    wo_r = wo.rearrange("(dc p) c -> p dc c", p=P)
    bq_r = bq.rearrange("(dc p) -> p dc", p=P)
    bk_r = bk.rearrange("(dc p) -> p dc", p=P)
    bo_r = bo.rearrange("(cc p) -> p cc", p=P)

    with contextlib.ExitStack() as ctx:
        singles = ctx.enter_context(tc.tile_pool(name="singles", bufs=1))
        qkpool = ctx.enter_context(tc.tile_pool(name="qk", bufs=2))
        etpool = ctx.enter_context(tc.tile_pool(name="et", bufs=8))
        rbpool = ctx.enter_context(tc.tile_pool(name="rb", bufs=4))
        tmppool = ctx.enter_context(tc.tile_pool(name="tmp", bufs=4))
        # PSUM bank budget (8 banks): energy 2 tiles x 2 banks, AV 1 x 2,
        # QK/vproj/warm (aux) 1 x 2. Keeping aux OUT of the energy pool means
        # the hoisted QK matmuls never displace the energy-tile rotation.
        pse = ctx.enter_context(tc.tile_pool(name="pse", bufs=2, space="PSUM"))
        psav = ctx.enter_context(tc.tile_pool(name="psav", bufs=1, space="PSUM"))
        psaux = ctx.enter_context(tc.tile_pool(name="psaux", bufs=1, space="PSUM"))

        # ---- input DMAs, spread across queues, first-needed first ----
        # sync: xb query-half 0; vector: xb half 1 (DVE idle at start)
        # scalar: bq/bk + wq/wk (t=0 columns first); gpsimd: bv/wv + bo/wo
        xb_sb = singles.tile([P, CC, S], F16)
        bq_sb = singles.tile([P, CC], F32)
        bk_sb = singles.tile([P, CC], F32)
        bo_sb = singles.tile([P, CC], F32)
        bv_sb = singles.tile([1, C], F16)
        wq_sb = singles.tile([P, CC, C], F16)
        wk_sb = singles.tile([P, CC, C], F16)
        wv_sb = singles.tile([P, CC, C], F16)
        wo_sb = singles.tile([P, CC, C], F16)

        # xb per-cc chunks (2KB contiguous rows) in the order the t=0 QK
        # accumulation consumes them; wq/wk per-cc interleaved likewise.
        for cc in range(CC):
            nc.sync.dma_start(out=xb_sb[:, cc], in_=xb_r[:, cc])
        nc.scalar.dma_start(out=wq_sb[:, 0], in_=wq_r[:, 0])
        nc.scalar.dma_start(out=wk_sb[:, 0], in_=wk_r[:, 0])
        nc.scalar.dma_start(out=bq_sb, in_=bq_r)
        nc.scalar.dma_start(out=bk_sb, in_=bk_r)
        for cc in range(1, CC):
            nc.scalar.dma_start(out=wq_sb[:, cc], in_=wq_r[:, cc])
            nc.scalar.dma_start(out=wk_sb[:, cc], in_=wk_r[:, cc])
        nc.gpsimd.dma_start(out=bv_sb, in_=bv[None, :])
        nc.gpsimd.dma_start(out=wv_sb, in_=wv_r)
        nc.gpsimd.dma_start(out=bo_sb, in_=bo_r)
        nc.gpsimd.dma_start(out=wo_sb, in_=wo_r)

        ones_row = singles.tile([1, P], F16)
        nc.vector.memset(ones_row.bitcast(U16), ONE_F16)

        # V' layout: 128 columns per (jc, h): [ones(64) | V(64)]. The AV matmul
        # (M=128) replicates Z = sum_j E[j, i] on psum rows 0..63 and puts O'
        # on rows 64..127 for every head. The ones halves are memset per key
        # chunk on GpSimd (idle engine; keeps the DVE queue clear for the t=0
        # bias adds); the V copies overwrite cols 64..127.
        Vp = singles.tile([P, SC, NH, P], F16)
        for sc in range(SC):
            nc.gpsimd.memset(Vp[:, sc, :, 0:64].bitcast(U16), ONE_F16)
        # bv replicated across partitions once (idle GpSimd) so the V bias is
        # folded into the psum->Vp copy instead of a per-chunk ones matmul
        bv_rep = singles.tile([P, C], F16)
        nc.gpsimd.partition_broadcast(bv_rep, bv_sb, channels=P)

        # PE warm-up on zeros while input DMAs land (HAM clock-gate at 8/8
        # before real matmuls start)
        warm = singles.tile([P, 512], F16)
        nc.vector.memset(warm.bitcast(U16), 0)
        ps_w = psaux.tile([P, 2, 512], F32, tag="aux")
        for _ in range(N_WARM):
            nc.tensor.matmul(ps_w[:, 0], warm[:, 0:128], warm)

        OTs = [singles.tile([P, S], F16, tag=f"ot{t}", name=f"ot{t}") for t in range(CC)]

        def emit_v_projection_chunk(sc):
            # V[s, d] = xr^T @ wv + bv for one token chunk
            ps_v = pse.tile([P, 2, 512], F32, tag="e")
            for cc in range(CC):
                nc.tensor.matmul(
                    ps_v[:, 0],
                    xb_sb[:, cc, sc * P : (sc + 1) * P],
                    wv_sb[:, cc],
                    start=(cc == 0),
                    stop=(cc == CC - 1),
                )
            psv_r = ps_v[:, 0].rearrange("p (h d) -> p h d", h=NH)
            bv_r = bv_rep.rearrange("p (h d) -> p h d", h=NH)
            nc.vector.tensor_tensor(Vp[:, sc, :, 64:128], psv_r, bv_r, ADD)

        pending_norm = [None]
        pending_av = []

        def flush_av(depth=0):
            while len(pending_av) > depth:
                pending_av.pop(0)()

        def flush_norm():
            if pending_norm[0] is not None:
                pending_norm[0]()
                pending_norm[0] = None

        qk_tiles = {}

        def make_qk_steps(tn):
            # Hoisted QK for head-pair tn, split in three: i0 matmuls; i0
            # bias + i1 matmuls; i1 bias. Spreading them keeps the 16-matmul
            # block off the head-pair seam (where it starved ScalarE for
            # ~5us) and keeps the DVE bias ops away from the DVE-exp unit.
            qt = qkpool.tile([P, S], F16, tag="qt", name=f"qt{tn}")
            kt = qkpool.tile([P, S], F16, tag="kt", name=f"kt{tn}")
            qk_tiles[tn] = (qt, kt)
            ps_list = []

            def mms(i):
                sl = slice(i * 512, (i + 1) * 512)
                ps_p = psaux.tile([P, 2, 512], F32, tag="aux")
                ps_list.append(ps_p)
                for cc in range(CC):
                    xsl = xb_sb[:, cc, sl]
                    nc.tensor.matmul(
                        ps_p[:, 0], wq_sb[:, cc, tn * P : (tn + 1) * P], xsl,
                        start=(cc == 0), stop=(cc == CC - 1),
                    )
                    nc.tensor.matmul(
                        ps_p[:, 1], wk_sb[:, cc, tn * P : (tn + 1) * P], xsl,
                        start=(cc == 0), stop=(cc == CC - 1),
                    )

            def bias(i):
                sl = slice(i * 512, (i + 1) * 512)
                nc.vector.tensor_scalar_add(
                    qt[:, sl], ps_list[i][:, 0], bq_sb[:, tn : tn + 1]
                )
                nc.vector.tensor_scalar_add(
                    kt[:, sl], ps_list[i][:, 1], bk_sb[:, tn : tn + 1]
                )

            return [
                lambda: mms(0),
                lambda: (bias(0), mms(1)),
                lambda: bias(1),
            ]

        def emit_qk(t):
            # QT/KT for heads (2t, 2t+1)
            qt = qkpool.tile([P, S], F16, tag="qt", name=f"qt{t}")
            kt = qkpool.tile([P, S], F16, tag="kt", name=f"kt{t}")
            qk_tiles[t] = (qt, kt)
            for i in range(NI):
                sl = slice(i * 512, (i + 1) * 512)
                ps_p = psaux.tile([P, 2, 512], F32, tag="aux")
                for cc in range(CC):
                    xsl = xb_sb[:, cc, sl]
                    nc.tensor.matmul(
                        ps_p[:, 0],
                        wq_sb[:, cc, t * P : (t + 1) * P],
                        xsl,
                        start=(cc == 0),
                        stop=(cc == CC - 1),
                    )
                    nc.tensor.matmul(
                        ps_p[:, 1],
                        wk_sb[:, cc, t * P : (t + 1) * P],
                        xsl,
                        start=(cc == 0),
                        stop=(cc == CC - 1),
                    )
                nc.vector.tensor_scalar_add(qt[:, sl], ps_p[:, 0], bq_sb[:, t : t + 1])
                nc.vector.tensor_scalar_add(kt[:, sl], ps_p[:, 1], bk_sb[:, t : t + 1])

        # ---- per head-pair t ----
        # QK for t+1 is emitted mid-way through t's first query-half, where it
        # fills the PE slack of the ScalarE-bound exp stream (instead of
        # stalling ACT for ~4.7us at every head-pair seam).
        emit_qk(0)
        for t in range(CC):
            qt, kt = qk_tiles.pop(t)
            # energy -> exp -> AV, pipelined per (query-half i, key-chunk jc).
            h0, h1 = 2 * t, 2 * t + 1
            for i in range(NI):
                sl = slice(i * 512, (i + 1) * 512)
                # The previous half's norm must be emitted to the DVE queue
                # before this half's DVE-exp units: with a single AV buffer,
                # an exp-DVE op queued ahead of the norm would deadlock the
                # PE FIFO (av waits buffer <- norm waits exp <- energy behind
                # the stalled av).
                flush_norm()
                ps_av = psav.tile([P, 2, 512], F32, tag="av")  # h0, h1
                for jc in range(SC):
                    if t == 0 and i == 0:
                        emit_v_projection_chunk(jc)
                    ih = 1 if t == 0 else 0
                    if i == ih and t < CC - 1:
                        if jc == 2:
                            qk_steps = make_qk_steps(t + 1)
                            qk_steps[0]()
                        elif jc == 5:
                            qk_steps[1]()
                        elif jc == 7:
                            qk_steps[2]()
                    k0 = kt[0:64, jc * P : (jc + 1) * P]
                    k1 = kt[64:128, jc * P : (jc + 1) * P]
                    first, last = jc == 0, jc == SC - 1
                    ps_e = pse.tile([P, 2, 512], F32, tag="e")  # head-major
                    nc.tensor.matmul(ps_e[:, 0], k0, qt[0:64, sl])
                    nc.tensor.matmul(ps_e[:, 1], k1, qt[64:128, sl])
                    et = etpool.tile([P, 2, 512], F16, tag="et")
                    if jc in _DVE_UNITS:
                        # Schraudolph f16 bit-pattern exp on DVE
                        nc.vector.tensor_scalar(
                            et.bitcast(U16), ps_e, SCH_A, SCH_B, MULT, ADD
                        )
                    else:
                        nc.scalar.activation(
                            out=et, in_=ps_e, func=EXP, scale=softmax_scale
                        )
                    # AV emitted one unit late so the in-order PE stream never
                    # waits on this unit's exp
                    flush_av(depth=1)

                    def av(ps_av=ps_av, jc=jc, et=et, h0=h0, h1=h1,
                           first=first, last=last):
                        nc.tensor.matmul(
                            ps_av[:, 0], Vp[:, jc, h0], et[:, 0],
                            start=first, stop=last,
                        )
                        nc.tensor.matmul(
                            ps_av[:, 1], Vp[:, jc, h1], et[:, 1],
                            start=first, stop=last,
                        )

                    pending_av.append(av)

                flush_av()

                def norm(t=t, sl=sl, ps_av=ps_av):
                    # Z replicated on psum rows 0..63 (ones half of V'); O' on
                    # rows 64..127. Reciprocal + multiply read partition-
                    # aligned rows; builtin DVE ops may cross partition bases.
                    if DIV_NORM:
                        nc.vector.tensor_tensor(
                            OTs[t][0:64, sl], ps_av[64:128, 0], ps_av[0:64, 0], DIV
                        )
                        nc.vector.tensor_tensor(
                            OTs[t][64:128, sl], ps_av[64:128, 1], ps_av[0:64, 1], DIV
                        )
                    else:
                        rb = rbpool.tile([64, 2, 512], F32, tag="rb")
                        nc.vector.reciprocal_approx_fast(out=rb, in_=ps_av[0:64])
                        nc.vector.tensor_tensor(
                            OTs[t][0:64, sl], ps_av[64:128, 0], rb[:, 0], MULT
                        )
                        nc.vector.tensor_tensor(
                            OTs[t][64:128, sl], ps_av[64:128, 1], rb[:, 1], MULT
                        )

                pending_norm[0] = norm

        # ---- final projection + bias + residual ----
        # All four cc accumulators live at once (two from each drained pool),
        # so the 32 matmuls run back-to-back; dc=3 waits only on the deferred
        # t=3 normalization, which overlaps dc=0..2.
        # cc=0..2 accumulators have no norm dependency (their psum buffers are
        # freed by exp / the early norm); cc=3's buffer waits on the deferred
        # t=3 i=1 norm, so its matmuls are emitted after 18 cover matmuls.
        flush_norm()
        ps_fs = [
            pse.tile([P, 2, 512], F32, tag="e", name="psf0"),
            pse.tile([P, 2, 512], F32, tag="e", name="psf1"),
            psaux.tile([P, 2, 512], F32, tag="aux", name="psf2"),
            psav.tile([P, 2, 512], F32, tag="av", name="psf3"),
        ]

        def fmm(dc, cc, start, stop):
            wo_sl = wo_sb[:, dc, cc * P : (cc + 1) * P]
            for i in range(NI):
                sl = slice(i * 512, (i + 1) * 512)
                nc.tensor.matmul(
                    ps_fs[cc][:, i], wo_sl, OTs[dc][:, sl], start=start, stop=stop,
                )

        for dc in range(CC - 1):
            for cc in range(CC - 1):
                fmm(dc, cc, dc == 0, False)
        for dc in range(CC - 1):
            fmm(dc, 3, dc == 0, False)
        out_q = [nc.sync, nc.scalar, nc.gpsimd]
        for cc in range(CC):
            fmm(CC - 1, cc, False, True)
            # per-half STT + store so DVE/DMA pipeline with the cc loop
            for i in range(NI):
                sl = slice(i * 512, (i + 1) * 512)
                tmp = tmppool.tile([P, 512], F16, tag="tmp")
                nc.vector.scalar_tensor_tensor(
                    out=tmp,
                    in0=ps_fs[cc][:, i],
                    scalar=bo_sb[:, cc : cc + 1],
                    in1=xb_sb[:, cc, sl],
                    op0=ADD,
                    op1=ADD,
                )
                out_q[(2 * cc + i) % 3].dma_start(out=y_r[:, cc, sl], in_=tmp)


_NC_CACHE = {}


def _build():
    key = (DVE_EXP, DIV_NORM, SCHRAUD_C, N_WARM)
    if key in _NC_CACHE:
        return _NC_CACHE[key]
    import concourse.bacc as bacc
    import concourse.mybir as mybir
    import concourse.tile as tile

    F32 = mybir.dt.float32
    F16 = mybir.dt.float16
    nc = bacc.Bacc("TRN2", target_bir_lowering=False, debug=False)
    aps = {}
    aps["xb"] = nc.dram_tensor("xb", (C, S), F16, kind="ExternalInput").ap()
    for name in ("wq", "wk", "wv", "wo"):
        aps[name] = nc.dram_tensor(name, (C, C), F16, kind="ExternalInput").ap()
    for name in ("bq", "bk", "bo"):
        aps[name] = nc.dram_tensor(name, (C,), F32, kind="ExternalInput").ap()
    aps["bv"] = nc.dram_tensor("bv", (C,), F16, kind="ExternalInput").ap()
    # y in f16: halves the output DMA tail; the extra ~5e-4 rounding is far
    # inside the tolerance. The host upcasts back to f32.
    aps["y"] = nc.dram_tensor("y", (C, S), F16, kind="ExternalOutput").ap()
    with tile.TileContext(nc) as tc:
        _emit(nc, tc, mybir, aps)
    nc.compile()
    _NC_CACHE[key] = nc
    return nc


def prepare_in_maps(x, wq, bq, wk, bk, wv, bv, wo, bo):
    """Host-side prep: cast activations/weights to f16, shard x per core."""
    x = np.asarray(x, dtype=np.float32).reshape(B, C, S)
    weights = {
        "wq": np.ascontiguousarray(np.asarray(wq, dtype=np.float16)),
        "bq": np.ascontiguousarray(np.asarray(bq, dtype=np.float32)),
        "wk": np.ascontiguousarray(np.asarray(wk, dtype=np.float16)),
        "bk": np.ascontiguousarray(np.asarray(bk, dtype=np.float32)),
        "wv": np.ascontiguousarray(np.asarray(wv, dtype=np.float16)),
        "bv": np.ascontiguousarray(np.asarray(bv, dtype=np.float16)),
        "wo": np.ascontiguousarray(np.asarray(wo, dtype=np.float16)),
        "bo": np.ascontiguousarray(np.asarray(bo, dtype=np.float32)),
    }
    return [
        {"xb": np.ascontiguousarray(x[b].astype(np.float16)), **weights}
        for b in range(B)
    ]


def kernel(x, wq, bq, wk, bk, wv, bv, wo, bo):
    from concourse import bass_utils

    nc = _build()
    in_maps = prepare_in_maps(x, wq, bq, wk, bk, wv, bv, wo, bo)
    res = bass_utils.run_bass_kernel_spmd(nc, in_maps, core_ids=list(range(B)))
    out = np.stack([r["y"].astype(np.float32) for r in res.results])
    return out.reshape(B, C, 32, 32)


# revision 37
# speedup vs baseline: 1.0720x; 1.0250x over previous
"""AttentionBlock Trainium2 kernel (v2).

Data-parallel: one batch element per NeuronCore (8 cores, no collectives).

Per core, with xr = x[b] viewed as [C, S] (C=512 channels, S=1024 tokens):
    QT = wq^T @ xr + bq   -> [D, S]  (d on partitions; head h = rows 64h..64h+63)
    KT = wk^T @ xr + bk   -> [D, S]
    V  = xr^T @ wv + bv   -> [S, D]  (tokens on partitions)
    per head h: ET[j, i] = KT_h^T . QT_h            (keys j on psum partitions)
                E = exp(ET / sqrt(C))               (no max-subtract; |args| < ~3)
                O'T[d, i] = sum_j V[j, d] E[j, i];  Z[i] = sum_j E[j, i]
                OT[d, i] = O'T[d, i] / Z[i]
    y = wo^T @ OT + bo + xr   -> [C, S]

All matmul operands are fp16 (cast host-side; DMA moves 2B/elem). PSUM stays
fp32. Heads run in pairs (2t, 2t+1): the pair's K rows live in SBUF partitions
0..63 / 64..127, so the two K=64 energy matmuls run concurrently in disjoint
PE row groups.

V' stationary layout per (jc, h): 128 columns = [ones(64) | V(64)]. The AV
matmul therefore lands Z = sum_j E[j, i] REPLICATED on psum partitions 0..63
(and O' on 64..127) for every head, so the softmax normalization is just
  rb[0:64, h] = 1/ps_av[0:64, h]   (DVE reciprocal, 64 lanes)
  OT[64h..]   = ps_av[64:128, h] * rb[0:64, h]
with no cross-partition broadcast (the old gpsimd partition_broadcast is gone).

exp is split between ScalarE (exact, table-based) and the Vector engine
(Schraudolph bit-trick: exp(s*x) ~= bitcast_f16(u16(x*A + B)), one full-rate
tensor_scalar op; |rel err| < ~2%, which drowns in the softmax average).
ATTN_DVE_EXP selects how many of the 8 key-chunk units per (t, i) go to DVE.

Pipelining: energy pair -> exp -> (AV deferred one unit so the in-order PE
stream never waits on this unit's exp). PSUM (8 banks): energy units
double-buffered (4), one AV accumulator (2), QK/vproj/warm on a separate aux
buffer (2) so the hoisted next-pair QK matmuls never displace the energy
rotation. The previous half's norm is flushed to the DVE queue at each half
start (required: with a single AV buffer, a DVE-exp op queued ahead of the
norm would deadlock the PE FIFO). QK for head-pair t+1 is emitted mid-way
through t's stream to fill PE slack; the final projection's cc=3 accumulator
(the one gated on the last norm) is emitted after 18 cover matmuls.
"""

import math
import os

import numpy as np

B = 8
C = 512
S = 1024  # 32*32 tokens
NH = 8
HD = 64
P = 128
CC = C // P  # 4 contraction chunks of 128
NI = 2  # S split into 2 chunks of 512 for matmul free dim
SC = S // P  # 8 key chunks of 128

DVE_EXP = int(os.environ.get("ATTN_DVE_EXP", "1"))  # of 8 units per (t, i)
DIV_NORM = os.environ.get("ATTN_DIV_NORM", "0") == "1"
SCHRAUD_C = float(os.environ.get("ATTN_SCHRAUD_C", "44.75"))
N_WARM = int(os.environ.get("ATTN_WARM", "16"))

# Which key-chunk units of each (t, i) run exp on DVE. Positioned mid-half,
# right after the previous half's norm chain (2.7us) drains from the DVE
# FIFO — an exp-DVE op queued behind other DVE work stalls the 2-deep
# energy-tile rotation and starves ScalarE.
_DVE_UNITS = {0: (), 1: (4,), 2: (4, 5), 3: (3, 4, 5), 4: (3, 4, 5, 6)}.get(
    DVE_EXP, tuple(range(DVE_EXP))
)


def _emit(nc, tc, mybir, aps):
    import contextlib

    F32 = mybir.dt.float32
    F16 = mybir.dt.float16
    U16 = mybir.dt.uint16
    MULT = mybir.AluOpType.mult
    ADD = mybir.AluOpType.add
    DIV = mybir.AluOpType.divide
    EXP = mybir.ActivationFunctionType.Exp
    softmax_scale = 1.0 / math.sqrt(C)
    ONE_F16 = 0x3C00

    # Schraudolph constants for f16 bit-pattern exp of (softmax_scale * x):
    # u16(x * scale * 1024/ln2 + (15*1024 - c))
    SCH_A = softmax_scale * 1024.0 / math.log(2.0)
    SCH_B = 15.0 * 1024.0 - SCHRAUD_C

    xb, wq, bq, wk, bk, wv, bv, wo, bo, y = (
        aps[k] for k in ("xb", "wq", "bq", "wk", "bk", "wv", "bv", "wo", "bo", "y")
    )
    xb_r = xb.rearrange("(cc p) s -> p cc s", p=P)
    y_r = y.rearrange("(cc p) s -> p cc s", p=P)
    wq_r = wq.rearrange("(cc p) d -> p cc d", p=P)
    wk_r = wk.rearrange("(cc p) d -> p cc d", p=P)
    wv_r = wv.rearrange("(cc p) d -> p cc d", p=P)
    wo_r = wo.rearrange("(dc p) c -> p dc c", p=P)
    bq_r = bq.rearrange("(dc p) -> p dc", p=P)
    bk_r = bk.rearrange("(dc p) -> p dc", p=P)
    bo_r = bo.rearrange("(cc p) -> p cc", p=P)

    with contextlib.ExitStack() as ctx:
        singles = ctx.enter_context(tc.tile_pool(name="singles", bufs=1))
        qkpool = ctx.enter_context(tc.tile_pool(name="qk", bufs=2))
        etpool = ctx.enter_context(tc.tile_pool(name="et", bufs=8))
        rbpool = ctx.enter_context(tc.tile_pool(name="rb", bufs=4))
        tmppool = ctx.enter_context(tc.tile_pool(name="tmp", bufs=4))
        # PSUM bank budget (8 banks): energy 2 tiles x 2 banks, AV 1 x 2,
        # QK/vproj/warm (aux) 1 x 2. Keeping aux OUT of the energy pool means
        # the hoisted QK matmuls never displace the energy-tile rotation.
        pse = ctx.enter_context(tc.tile_pool(name="pse", bufs=2, space="PSUM"))
        psav = ctx.enter_context(tc.tile_pool(name="psav", bufs=1, space="PSUM"))
        psaux = ctx.enter_context(tc.tile_pool(name="psaux", bufs=1, space="PSUM"))

        # ---- input DMAs, spread across queues, first-needed first ----
        # sync: xb query-half 0; vector: xb half 1 (DVE idle at start)
        # scalar: bq/bk + wq/wk (t=0 columns first); gpsimd: bv/wv + bo/wo
        xb_sb = singles.tile([P, CC, S], F16)
        bq_sb = singles.tile([P, CC], F32)
        bk_sb = singles.tile([P, CC], F32)
        bo_sb = singles.tile([P, CC], F32)
        bv_sb = singles.tile([1, C], F16)
        wq_sb = singles.tile([P, CC, C], F16)
        wk_sb = singles.tile([P, CC, C], F16)
        wv_sb = singles.tile([P, CC, C], F16)
        wo_sb = singles.tile([P, CC, C], F16)

        # The critical start chain is QK(t=0) -> first energies. Ship ONLY the
        # t=0 weight columns first (256KB instead of 1MB), then wv (gates the
        # vproj units), then the remaining columns (first needed by the QK
        # hoist at ~40us). xb cc3 rides the otherwise-idle gpsimd queue.
        for cc in range(CC - 1):
            nc.sync.dma_start(out=xb_sb[:, cc], in_=xb_r[:, cc])
        nc.scalar.dma_start(out=wq_sb[:, :, 0:128], in_=wq_r[:, :, 0:128])
        nc.scalar.dma_start(out=wk_sb[:, :, 0:128], in_=wk_r[:, :, 0:128])
        nc.scalar.dma_start(out=bq_sb, in_=bq_r)
        nc.scalar.dma_start(out=bk_sb, in_=bk_r)
        nc.scalar.dma_start(out=wv_sb, in_=wv_r)
        nc.scalar.dma_start(out=wq_sb[:, :, 128:512], in_=wq_r[:, :, 128:512])
        nc.scalar.dma_start(out=wk_sb[:, :, 128:512], in_=wk_r[:, :, 128:512])
        nc.gpsimd.dma_start(out=xb_sb[:, 3], in_=xb_r[:, 3])
        nc.gpsimd.dma_start(out=bv_sb, in_=bv[None, :])
        nc.gpsimd.dma_start(out=bo_sb, in_=bo_r)
        nc.gpsimd.dma_start(out=wo_sb, in_=wo_r)

        ones_row = singles.tile([1, P], F16)
        nc.vector.memset(ones_row.bitcast(U16), ONE_F16)

        # V' layout: 128 columns per (jc, h): [ones(64) | V(64)]. The AV matmul
        # (M=128) replicates Z = sum_j E[j, i] on psum rows 0..63 and puts O'
        # on rows 64..127 for every head. The ones halves are memset per key
        # chunk on GpSimd (idle engine; keeps the DVE queue clear for the t=0
        # bias adds); the V copies overwrite cols 64..127.
        Vp = singles.tile([P, SC, NH, P], F16)
        for sc in range(SC):
            nc.gpsimd.memset(Vp[:, sc, :, 0:64].bitcast(U16), ONE_F16)
        # bv replicated across partitions once (idle GpSimd) so the V bias is
        # folded into the psum->Vp copy instead of a per-chunk ones matmul
        bv_rep = singles.tile([P, C], F16)
        nc.gpsimd.partition_broadcast(bv_rep, bv_sb, channels=P)

        # PE warm-up on zeros while input DMAs land (HAM clock-gate at 8/8
        # before real matmuls start)
        warm = singles.tile([P, 512], F16)
        nc.vector.memset(warm.bitcast(U16), 0)
        ps_w = psaux.tile([P, 2, 512], F32, tag="aux")
        for _ in range(N_WARM):
            nc.tensor.matmul(ps_w[:, 0], warm[:, 0:128], warm)

        OTs = [singles.tile([P, S], F16, tag=f"ot{t}", name=f"ot{t}") for t in range(CC)]

        def emit_v_projection_chunk(sc):
            # V[s, d] = xr^T @ wv + bv for one token chunk
            ps_v = pse.tile([P, 2, 512], F32, tag="e")
            for cc in range(CC):
                nc.tensor.matmul(
                    ps_v[:, 0],
                    xb_sb[:, cc, sc * P : (sc + 1) * P],
                    wv_sb[:, cc],
                    start=(cc == 0),
                    stop=(cc == CC - 1),
                )
            psv_r = ps_v[:, 0].rearrange("p (h d) -> p h d", h=NH)
            bv_r = bv_rep.rearrange("p (h d) -> p h d", h=NH)
            nc.vector.tensor_tensor(Vp[:, sc, :, 64:128], psv_r, bv_r, ADD)

        pending_norm = [None]
        pending_av = []

        def flush_av(depth=0):
            while len(pending_av) > depth:
                pending_av.pop(0)()

        def flush_norm():
            if pending_norm[0] is not None:
                pending_norm[0]()
                pending_norm[0] = None

        qk_tiles = {}

        def make_qk_steps(tn):
            # Hoisted QK for head-pair tn, split in three: i0 matmuls; i0
            # bias + i1 matmuls; i1 bias. Spreading them keeps the 16-matmul
            # block off the head-pair seam (where it starved ScalarE for
            # ~5us) and keeps the DVE bias ops away from the DVE-exp unit.
            qt = qkpool.tile([P, S], F16, tag="qt", name=f"qt{tn}")
            kt = qkpool.tile([P, S], F16, tag="kt", name=f"kt{tn}")
            qk_tiles[tn] = (qt, kt)
            ps_list = []

            def mms(i):
                sl = slice(i * 512, (i + 1) * 512)
                ps_p = psaux.tile([P, 2, 512], F32, tag="aux")
                ps_list.append(ps_p)
                for cc in range(CC):
                    xsl = xb_sb[:, cc, sl]
                    nc.tensor.matmul(
                        ps_p[:, 0], wq_sb[:, cc, tn * P : (tn + 1) * P], xsl,
                        start=(cc == 0), stop=(cc == CC - 1),
                    )
                    nc.tensor.matmul(
                        ps_p[:, 1], wk_sb[:, cc, tn * P : (tn + 1) * P], xsl,
                        start=(cc == 0), stop=(cc == CC - 1),
                    )

            def bias(i):
                sl = slice(i * 512, (i + 1) * 512)
                nc.vector.tensor_scalar_add(
                    qt[:, sl], ps_list[i][:, 0], bq_sb[:, tn : tn + 1]
                )
                nc.vector.tensor_scalar_add(
                    kt[:, sl], ps_list[i][:, 1], bk_sb[:, tn : tn + 1]
                )

            return [
                lambda: mms(0),
                lambda: (bias(0), mms(1)),
                lambda: bias(1),
            ]

        def emit_qk(t):
            # QT/KT for heads (2t, 2t+1)
            qt = qkpool.tile([P, S], F16, tag="qt", name=f"qt{t}")
            kt = qkpool.tile([P, S], F16, tag="kt", name=f"kt{t}")
            qk_tiles[t] = (qt, kt)
            for i in range(NI):
                sl = slice(i * 512, (i + 1) * 512)
                ps_p = psaux.tile([P, 2, 512], F32, tag="aux")
                for cc in range(CC):
                    xsl = xb_sb[:, cc, sl]
                    nc.tensor.matmul(
                        ps_p[:, 0],
                        wq_sb[:, cc, t * P : (t + 1) * P],
                        xsl,
                        start=(cc == 0),
                        stop=(cc == CC - 1),
                    )
                    nc.tensor.matmul(
                        ps_p[:, 1],
                        wk_sb[:, cc, t * P : (t + 1) * P],
                        xsl,
                        start=(cc == 0),
                        stop=(cc == CC - 1),
                    )
                nc.vector.tensor_scalar_add(qt[:, sl], ps_p[:, 0], bq_sb[:, t : t + 1])
                nc.vector.tensor_scalar_add(kt[:, sl], ps_p[:, 1], bk_sb[:, t : t + 1])

        # ---- per head-pair t ----
        # QK for t+1 is emitted mid-way through t's first query-half, where it
        # fills the PE slack of the ScalarE-bound exp stream (instead of
        # stalling ACT for ~4.7us at every head-pair seam).
        emit_qk(0)
        for t in range(CC):
            qt, kt = qk_tiles.pop(t)
            # energy -> exp -> AV, pipelined per (query-half i, key-chunk jc).
            h0, h1 = 2 * t, 2 * t + 1
            for i in range(NI):
                sl = slice(i * 512, (i + 1) * 512)
                # The previous half's norm must be emitted to the DVE queue
                # before this half's DVE-exp units: with a single AV buffer,
                # an exp-DVE op queued ahead of the norm would deadlock the
                # PE FIFO (av waits buffer <- norm waits exp <- energy behind
                # the stalled av).
                flush_norm()
                ps_av = psav.tile([P, 2, 512], F32, tag="av")  # h0, h1
                for jc in range(SC):
                    ih = 1 if t == 0 else 0
                    if i == ih and t < CC - 1:
                        if jc == 2:
                            qk_steps = make_qk_steps(t + 1)
                            qk_steps[0]()
                        elif jc == 5:
                            qk_steps[1]()
                        elif jc == 7:
                            qk_steps[2]()
                    k0 = kt[0:64, jc * P : (jc + 1) * P]
                    k1 = kt[64:128, jc * P : (jc + 1) * P]
                    first, last = jc == 0, jc == SC - 1
                    ps_e = pse.tile([P, 2, 512], F32, tag="e")  # head-major
                    nc.tensor.matmul(ps_e[:, 0], k0, qt[0:64, sl])
                    nc.tensor.matmul(ps_e[:, 1], k1, qt[64:128, sl])
                    if t == 0 and i == 0:
                        emit_v_projection_chunk(jc)
                    et = etpool.tile([P, 2, 512], F16, tag="et")
                    if jc in _DVE_UNITS:
                        # Schraudolph f16 bit-pattern exp on DVE
                        nc.vector.tensor_scalar(
                            et.bitcast(U16), ps_e, SCH_A, SCH_B, MULT, ADD
                        )
                    else:
                        nc.scalar.activation(
                            out=et, in_=ps_e, func=EXP, scale=softmax_scale
                        )
                    # AV emitted one unit late so the in-order PE stream never
                    # waits on this unit's exp
                    flush_av(depth=1)

                    def av(ps_av=ps_av, jc=jc, et=et, h0=h0, h1=h1,
                           first=first, last=last):
                        nc.tensor.matmul(
                            ps_av[:, 0], Vp[:, jc, h0], et[:, 0],
                            start=first, stop=last,
                        )
                        nc.tensor.matmul(
                            ps_av[:, 1], Vp[:, jc, h1], et[:, 1],
                            start=first, stop=last,
                        )

                    pending_av.append(av)

                flush_av()

                def norm(t=t, sl=sl, ps_av=ps_av):
                    # Z replicated on psum rows 0..63 (ones half of V'); O' on
                    # rows 64..127. Reciprocal + multiply read partition-
                    # aligned rows; builtin DVE ops may cross partition bases.
                    if DIV_NORM:
                        nc.vector.tensor_tensor(
                            OTs[t][0:64, sl], ps_av[64:128, 0], ps_av[0:64, 0], DIV
                        )
                        nc.vector.tensor_tensor(
                            OTs[t][64:128, sl], ps_av[64:128, 1], ps_av[0:64, 1], DIV
                        )
                    else:
                        rb = rbpool.tile([64, 2, 512], F32, tag="rb")
                        nc.vector.reciprocal_approx_fast(out=rb, in_=ps_av[0:64])
                        nc.vector.tensor_tensor(
                            OTs[t][0:64, sl], ps_av[64:128, 0], rb[:, 0], MULT
                        )
                        nc.vector.tensor_tensor(
                            OTs[t][64:128, sl], ps_av[64:128, 1], rb[:, 1], MULT
                        )

                pending_norm[0] = norm

        # ---- final projection + bias + residual ----
        # All four cc accumulators live at once (two from each drained pool),
        # so the 32 matmuls run back-to-back; dc=3 waits only on the deferred
        # t=3 normalization, which overlaps dc=0..2.
        # cc=0..2 accumulators have no norm dependency (their psum buffers are
        # freed by exp / the early norm); cc=3's buffer waits on the deferred
        # t=3 i=1 norm, so its matmuls are emitted after 18 cover matmuls.
        flush_norm()
        ps_fs = [
            pse.tile([P, 2, 512], F32, tag="e", name="psf0"),
            pse.tile([P, 2, 512], F32, tag="e", name="psf1"),
            psaux.tile([P, 2, 512], F32, tag="aux", name="psf2"),
            psav.tile([P, 2, 512], F32, tag="av", name="psf3"),
        ]

        def fmm(dc, cc, start, stop):
            wo_sl = wo_sb[:, dc, cc * P : (cc + 1) * P]
            for i in range(NI):
                sl = slice(i * 512, (i + 1) * 512)
                nc.tensor.matmul(
                    ps_fs[cc][:, i], wo_sl, OTs[dc][:, sl], start=start, stop=stop,
                )

        for dc in range(CC - 1):
            for cc in range(CC - 1):
                fmm(dc, cc, dc == 0, False)
        for dc in range(CC - 1):
            fmm(dc, 3, dc == 0, False)
        out_q = [nc.sync, nc.scalar, nc.gpsimd]
        for cc in range(CC):
            fmm(CC - 1, cc, False, True)
            # per-half STT + store so DVE/DMA pipeline with the cc loop
            for i in range(NI):
                sl = slice(i * 512, (i + 1) * 512)
                tmp = tmppool.tile([P, 512], F16, tag="tmp")
                nc.vector.scalar_tensor_tensor(
                    out=tmp,
                    in0=ps_fs[cc][:, i],
                    scalar=bo_sb[:, cc : cc + 1],
                    in1=xb_sb[:, cc, sl],
                    op0=ADD,
                    op1=ADD,
                )
                out_q[(2 * cc + i) % 3].dma_start(out=y_r[:, cc, sl], in_=tmp)


_NC_CACHE = {}


def _build():
    key = (DVE_EXP, DIV_NORM, SCHRAUD_C, N_WARM)
    if key in _NC_CACHE:
        return _NC_CACHE[key]
    import concourse.bacc as bacc
    import concourse.mybir as mybir
    import concourse.tile as tile

    F32 = mybir.dt.float32
    F16 = mybir.dt.float16
    nc = bacc.Bacc("TRN2", target_bir_lowering=False, debug=False)
    aps = {}
    aps["xb"] = nc.dram_tensor("xb", (C, S), F16, kind="ExternalInput").ap()
    for name in ("wq", "wk", "wv", "wo"):
        aps[name] = nc.dram_tensor(name, (C, C), F16, kind="ExternalInput").ap()
    for name in ("bq", "bk", "bo"):
        aps[name] = nc.dram_tensor(name, (C,), F32, kind="ExternalInput").ap()
    aps["bv"] = nc.dram_tensor("bv", (C,), F16, kind="ExternalInput").ap()
    # y in f16: halves the output DMA tail; the extra ~5e-4 rounding is far
    # inside the tolerance. The host upcasts back to f32.
    aps["y"] = nc.dram_tensor("y", (C, S), F16, kind="ExternalOutput").ap()
    with tile.TileContext(nc) as tc:
        _emit(nc, tc, mybir, aps)
    nc.compile()
    _NC_CACHE[key] = nc
    return nc


def prepare_in_maps(x, wq, bq, wk, bk, wv, bv, wo, bo):
    """Host-side prep: cast activations/weights to f16, shard x per core."""
    x = np.asarray(x, dtype=np.float32).reshape(B, C, S)
    weights = {
        "wq": np.ascontiguousarray(np.asarray(wq, dtype=np.float16)),
        "bq": np.ascontiguousarray(np.asarray(bq, dtype=np.float32)),
        "wk": np.ascontiguousarray(np.asarray(wk, dtype=np.float16)),
        "bk": np.ascontiguousarray(np.asarray(bk, dtype=np.float32)),
        "wv": np.ascontiguousarray(np.asarray(wv, dtype=np.float16)),
        "bv": np.ascontiguousarray(np.asarray(bv, dtype=np.float16)),
        "wo": np.ascontiguousarray(np.asarray(wo, dtype=np.float16)),
        "bo": np.ascontiguousarray(np.asarray(bo, dtype=np.float32)),
    }
    return [
        {"xb": np.ascontiguousarray(x[b].astype(np.float16)), **weights}
        for b in range(B)
    ]


def kernel(x, wq, bq, wk, bk, wv, bv, wo, bo):
    from concourse import bass_utils

    nc = _build()
    in_maps = prepare_in_maps(x, wq, bq, wk, bk, wv, bv, wo, bo)
    res = bass_utils.run_bass_kernel_spmd(nc, in_maps, core_ids=list(range(B)))
    out = np.stack([r["y"].astype(np.float32) for r in res.results])
    return out.reshape(B, C, 32, 32)
